# revision 1
# baseline (speedup 1.0000x reference)
"""3-layer GAT on 8 Trainium2 NeuronCores (Bass/Tile).

Strategy: partition nodes across the 8 cores (graph parallel); edges live with
their destination core so segment-softmax/aggregation stay local; per layer,
all-gather the (bf16) node features + attention source/dest logits; gather
source rows per edge chunk with dma_gather; aggregate with selection-matrix
matmuls on the PE.

Self-contained: only imports the system concourse install.
"""

import os
import sys

for _p in ("/opt/trn_rl_repo", "/root/.axon_site/_ro/trn_rl_repo"):
    if os.path.isdir(_p) and _p not in sys.path:
        sys.path.insert(0, _p)

import math
from dataclasses import dataclass, field

import ml_dtypes
import numpy as np

import concourse.bacc as bacc
import concourse.bass as bass
import concourse.tile as tile
from concourse import mybir
from concourse.bass_utils import run_bass_kernel_spmd

P = 128
BF16 = mybir.dt.bfloat16
F32 = mybir.dt.float32
I16 = mybir.dt.int16
AL = mybir.AluOpType
AF = mybir.ActivationFunctionType

NEG_SLOPE_ATT = 0.2
NEG_SLOPE_ACT = 0.01
LN_EPS = 1e-5


def _ceil(a, b):
    return -(-a // b)


def _pad_elem(n_f32_elems):
    """bf16 row length (elements) padded so row bytes are a multiple of 256."""
    return _ceil(n_f32_elems * 2, 256) * 128


@dataclass
class Cfg:
    N: int = 50000
    E: int = 400000
    F_IN: int = 256
    HEADS: int = 4
    C1: int = 256
    C2: int = 128
    NCLS: int = 32
    NCORES: int = 8

    def __post_init__(self):
        assert self.N % self.NCORES == 0
        self.NL = self.N // self.NCORES
        self.T = _ceil(self.NL, P)
        self.NLP = self.T * P
        self.NPTOT = self.NLP * self.NCORES
        assert self.NPTOT % 2 == 0
        self.HALF = self.NPTOT // 2
        assert self.HALF <= 32767, "half-table must be int16 addressable"
        H = self.HEADS
        self.CO1 = H * self.C1
        self.CO2 = H * self.C2
        assert self.F_IN % P == 0 and self.CO1 % P == 0 and self.CO2 % P == 0
        # bf16 hs-table rows: [h | s | pad], bytes % 256 == 0
        self.ELEM1 = _pad_elem(self.CO1 + H)
        self.ELEM2 = _pad_elem(self.CO2 + H)
        self.ELEM3 = _pad_elem(self.NCLS + 1)


@dataclass
class Meta:
    nch: list  # [T][2] chunk counts (common across cores)
    si: list   # [T][2] idx16 column offsets
    sc: list   # [T][2] dstloc column offsets
    SI: int
    SC: int
    sd: list = None   # [T] dst-idx column offsets (per-tile d gather)
    SD: int = 0


def host_prep(cfg: Cfg, x, edge_src, edge_dst,
              W1, a_src1, a_dst1, b1, ln1_g, ln1_b,
              W2, a_src2, a_dst2, b2, ln2_g, ln2_b,
              W3, a_src3, a_dst3, b3, ln3_g, ln3_b):
    """Build per-core input maps + the (common) chunk structure."""
    c = cfg
    bf = ml_dtypes.bfloat16

    # ---- append self loops, shard edges by destination core
    loops = np.arange(c.N, dtype=np.int64)
    src = np.concatenate([edge_src.astype(np.int64), loops])
    dst = np.concatenate([edge_dst.astype(np.int64), loops])

    dst_core = dst // c.NL
    dstloc = dst - dst_core * c.NL
    tile_id = dstloc // P
    gsrc = (src // c.NL) * c.NLP + (src % c.NL)     # padded-global source row
    half = (gsrc >= c.HALF).astype(np.int64)
    idx16 = (gsrc - half * c.HALF).astype(np.int64)

    # group edges per (core, tile, half)
    counts = np.zeros((c.NCORES, c.T, 2), np.int64)
    np.add.at(counts, (dst_core, tile_id, half), 1)
    nch = np.maximum(_ceil_arr(counts.max(axis=0), P), 0)  # [T,2] chunks
    # offsets
    si = np.zeros((c.T, 2), np.int64)
    sc = np.zeros((c.T, 2), np.int64)
    acc_si = acc_sc = 0
    for t in range(c.T):
        for h in range(2):
            si[t, h] = acc_si
            sc[t, h] = acc_sc
            acc_si += int(nch[t, h]) * (P // 16)
            acc_sc += int(nch[t, h])
    SI, SC = int(acc_si), int(acc_sc)
    sd = np.zeros(c.T, np.int64)
    acc_sd = 0
    for t in range(c.T):
        sd[t] = acc_sd
        acc_sd += int(nch[t, 0] + nch[t, 1]) * (P // 16)
    SD = int(acc_sd)
    meta = Meta(nch=nch.tolist(), si=si.tolist(), sc=sc.tolist(), SI=SI, SC=SC,
                sd=sd.tolist(), SD=SD)

    # ---- per-core index / dstloc tables
    order = np.lexsort((half, tile_id, dst_core))  # group by (core, tile, half)
    src_s, half_s, t_s, core_s = (idx16[order], half[order], tile_id[order],
                                  dst_core[order])
    dstrel_s = (dstloc - tile_id * P)[order]

    idx_tabs, dl_tabs = [], []
    # per-core group start offsets
    starts = np.zeros((c.NCORES, c.T, 2), np.int64)
    run = 0
    for cc in range(c.NCORES):
        for t in range(c.T):
            for h in range(2):
                starts[cc, t, h] = run
                run += int(counts[cc, t, h])
    dstidx_tabs = []
    for cc in range(c.NCORES):
        itab = np.zeros((16, SI), np.int16)
        dtab = np.full((P, SC), -1.0, np.float32)
        ditab = np.zeros((16, SD), np.int16)
        for t in range(c.T):
            dchunk = 0  # chunk index within the tile (across halves)
            for h in range(2):
                m = int(counts[cc, t, h])
                n = int(nch[t, h])
                if n == 0:
                    continue
                s0 = int(starts[cc, t, h])
                iv = np.zeros(n * P, np.int16)
                iv[:m] = src_s[s0:s0 + m].astype(np.int16)
                cols = int(si[t, h])
                blk = iv.reshape(n * P // 16, 16).T  # idx k -> [k%16, k//16]
                itab[:, cols:cols + n * (P // 16)] = blk
                dv = np.full(n * P, -1.0, np.float32)
                dv[:m] = dstrel_s[s0:s0 + m].astype(np.float32)
                dtab[:, sc[t, h]:sc[t, h] + n] = dv.reshape(n, P).T
                # dst-row indices (into the local [NLP] d table)
                div = np.zeros(n * P, np.int16)
                div[:m] = (t * P + dstrel_s[s0:s0 + m]).astype(np.int16)
                dc = int(sd[t]) + dchunk * (P // 16)
                ditab[:, dc:dc + n * (P // 16)] = div.reshape(
                    n * P // 16, 16).T
                dchunk += n
        idx_tabs.append(np.tile(itab, (8, 1)))
        dl_tabs.append(dtab)
        dstidx_tabs.append(np.tile(ditab, (8, 1)))

    # ---- weights (augmented with U = W.T @ a columns), bf16
    def aug(W, a_s, a_d, H, C):
        WT = W.T.astype(np.float64)                      # [Fin, H*C]
        U_s = np.zeros((WT.shape[0], H))
        U_d = np.zeros((WT.shape[0], H))
        for h in range(H):
            U_s[:, h] = WT[:, h * C:(h + 1) * C] @ a_s[h].astype(np.float64)
            U_d[:, h] = WT[:, h * C:(h + 1) * C] @ a_d[h].astype(np.float64)
        return np.concatenate([WT, U_s, U_d], axis=1).astype(bf)

    W1a = aug(W1, a_src1, a_dst1, c.HEADS, c.C1)   # [F_IN, CO1+2H]
    W2a = aug(W2, a_src2, a_dst2, c.HEADS, c.C2)   # [CO1, CO2+2H]
    W3a = aug(W3, a_src3, a_dst3, 1, c.NCLS)       # [CO2, NCLS+2]

    def bln(b, g, be, D):
        row = np.concatenate([b, g, be]).astype(np.float32)[None, :]
        return np.repeat(row, P, axis=0)           # [128, 3D]

    bln1 = bln(b1, ln1_g, ln1_b, c.CO1)
    bln2 = bln(b2, ln2_g, ln2_b, c.CO2)
    bln3 = bln(b3, ln3_g, ln3_b, c.NCLS)

    iota_f = np.repeat(np.arange(P, dtype=np.float32)[None, :], P, axis=0)
    ident = np.eye(P, dtype=bf)

    in_maps = []
    for cc in range(c.NCORES):
        xl = np.zeros((c.NLP, c.F_IN), np.float32)
        xl[:c.NL] = x[cc * c.NL:(cc + 1) * c.NL]
        in_maps.append({
            "xT": np.ascontiguousarray(xl.T).astype(bf),
            "W1a": W1a, "W2a": W2a, "W3a": W3a,
            "bln1": bln1, "bln2": bln2, "bln3": bln3,
            "idx16": idx_tabs[cc], "dstloc": dl_tabs[cc],
            "dstidx16": dstidx_tabs[cc],
            "iotaf": iota_f, "ident": ident,
        })
    return in_maps, meta


def _ceil_arr(a, b):
    return -(-a // b)


# --------------------------------------------------------------------------
# device program
# --------------------------------------------------------------------------

def build_nc(cfg: Cfg, meta: Meta, max_phase: int = 6):
    c = cfg
    H = c.HEADS
    nc = bacc.Bacc("TRN2", target_bir_lowering=False, debug=False,
                   num_devices=c.NCORES, enable_partition_id=False)

    # ---- I/O
    xT = nc.dram_tensor("xT", [c.F_IN, c.NLP], BF16, kind="ExternalInput").ap()
    W1a = nc.dram_tensor("W1a", [c.F_IN, c.CO1 + 2 * H], BF16, kind="ExternalInput").ap()
    W2a = nc.dram_tensor("W2a", [c.CO1, c.CO2 + 2 * H], BF16, kind="ExternalInput").ap()
    W3a = nc.dram_tensor("W3a", [c.CO2, c.NCLS + 2], BF16, kind="ExternalInput").ap()
    bln1 = nc.dram_tensor("bln1", [P, 3 * c.CO1], F32, kind="ExternalInput").ap()
    bln2 = nc.dram_tensor("bln2", [P, 3 * c.CO2], F32, kind="ExternalInput").ap()
    bln3 = nc.dram_tensor("bln3", [P, 3 * c.NCLS], F32, kind="ExternalInput").ap()
    idx16 = nc.dram_tensor("idx16", [P, meta.SI], I16, kind="ExternalInput").ap()
    dstloc = nc.dram_tensor("dstloc", [P, meta.SC], F32, kind="ExternalInput").ap()
    dstidx16 = nc.dram_tensor("dstidx16", [P, meta.SD], I16,
                              kind="ExternalInput").ap()
    iotaf = nc.dram_tensor("iotaf", [P, P], F32, kind="ExternalInput").ap()
    ident = nc.dram_tensor("ident", [P, P], BF16, kind="ExternalInput").ap()
    y = nc.dram_tensor("y", [c.NLP, c.NCLS], F32, kind="ExternalOutput").ap()

    groups = [list(range(c.NCORES))]

    with tile.TileContext(nc) as tc:
        # ---- persistent DRAM intermediates
        dram_cm = tc.tile_pool(name="dram", bufs=1, space="DRAM")
        dram = dram_cm.__enter__()
        aspace = "Shared" if c.NCORES > 4 else "Local"
        hs1_loc = dram.tile([c.NLP, c.ELEM1], BF16)
        hs1_full = dram.tile([c.NPTOT, c.ELEM1], BF16, addr_space=aspace)
        hs2_loc = dram.tile([c.NLP, c.ELEM2], BF16)
        hs2_full = dram.tile([c.NPTOT, c.ELEM2], BF16, addr_space=aspace)
        hs3_loc = dram.tile([c.NLP, c.ELEM3], BF16)
        hs3_full = dram.tile([c.NPTOT, c.ELEM3], BF16, addr_space=aspace)
        # d tables with 256B rows for dma_gather (cols 0:H hold d, rest pad)
        d1t = dram.tile([c.NLP, P], BF16)
        d2t = dram.tile([c.NLP, P], BF16)
        d3t = dram.tile([c.NLP, P], BF16)

        # ---- persistent SBUF constants
        cpool_cm = tc.tile_pool(name="const", bufs=1)
        cpool = cpool_cm.__enter__()
        KC1 = c.F_IN // P
        W1w = c.CO1 + 2 * H
        W1a_sb = cpool.tile([P, KC1 * W1w], BF16)
        for k in range(KC1):
            nc.sync.dma_start(W1a_sb[:, k * W1w:(k + 1) * W1w],
                              W1a[k * P:(k + 1) * P, :])
        KC2 = c.CO1 // P
        W2w = c.CO2 + 2 * H
        W2a_sb = cpool.tile([P, KC2 * W2w], BF16)
        for k in range(KC2):
            nc.sync.dma_start(W2a_sb[:, k * W2w:(k + 1) * W2w],
                              W2a[k * P:(k + 1) * P, :])
        KC3 = c.CO2 // P
        W3w = c.NCLS + 2
        W3a_sb = cpool.tile([P, KC3 * W3w], BF16)
        for k in range(KC3):
            nc.sync.dma_start(W3a_sb[:, k * W3w:(k + 1) * W3w],
                              W3a[k * P:(k + 1) * P, :])
        bln1_sb = cpool.tile([P, 3 * c.CO1], F32)
        nc.sync.dma_start(bln1_sb[:], bln1[:])
        bln2_sb = cpool.tile([P, 3 * c.CO2], F32)
        nc.sync.dma_start(bln2_sb[:], bln2[:])
        bln3_sb = cpool.tile([P, 3 * c.NCLS], F32)
        nc.sync.dma_start(bln3_sb[:], bln3[:])
        idx_sb = cpool.tile([P, meta.SI], I16)
        nc.sync.dma_start(idx_sb[:], idx16[:])
        dl_sb = cpool.tile([P, meta.SC], F32)
        nc.sync.dma_start(dl_sb[:], dstloc[:])
        didx_sb = cpool.tile([P, meta.SD], I16)
        nc.sync.dma_start(didx_sb[:], dstidx16[:])
        iota_sb = cpool.tile([P, P], F32)
        nc.sync.dma_start(iota_sb[:], iotaf[:])
        id_sb = cpool.tile([P, P], BF16)
        nc.sync.dma_start(id_sb[:], ident[:])

        # ================= phase A: h1 = x @ W1a (per local node tile)
        with (
            tc.tile_pool(name="pA", bufs=3) as pA,
            tc.tile_pool(name="pAp", bufs=2, space="PSUM") as pAp,
        ):
            for t in range(c.T):
                xt = pA.tile([P, KC1 * P], BF16, tag="xt")
                for k in range(KC1):
                    nc.sync.dma_start(xt[:, k * P:(k + 1) * P],
                                      xT[k * P:(k + 1) * P, t * P:(t + 1) * P])
                hp = pAp.tile([P, W1w], F32, tag="hp")
                _mm_splits(nc, hp, xt, W1a_sb, KC1, W1w, P)
                _store_hs(nc, pA, hp, c.CO1, H, c.ELEM1, hs1_loc, d1t, t)

        if c.NCORES == 1:
            # single-core profiling build: tables are just the local shard
            hs1_full, hs2_full, hs3_full = hs1_loc, hs2_loc, hs3_loc
        if max_phase >= 1 and c.NCORES > 1:
            nc.gpsimd.collective_compute(
                "AllGather", AL.bypass, replica_groups=groups,
                ins=[hs1_loc[:].opt()], outs=[hs1_full[:].opt()])

        # ================= phase C: layer-1 aggregation + LN + fused L2 matmul
        if max_phase >= 2:
            _edge_phase(
                nc, tc, c, meta, lay=1, Hn=H, Ch=c.C1, ELEM=c.ELEM1,
                hs_full=hs1_full, d_tab=d1t, bln_sb=bln1_sb,
                iota_sb=iota_sb, id_sb=id_sb, idx_sb=idx_sb, dl_sb=dl_sb,
                didx_sb=didx_sb,
                fuse=dict(W_sb=W2a_sb, KC=KC2, Ww=W2w, CO=c.CO2, Hn2=H,
                          ELEMn=c.ELEM2, hs_loc=hs2_loc, d_next=d2t),
                final=None, y=None)

        if max_phase >= 3 and c.NCORES > 1:
            nc.gpsimd.collective_compute(
                "AllGather", AL.bypass, replica_groups=groups,
                ins=[hs2_loc[:].opt()], outs=[hs2_full[:].opt()])

        # ================= phase E: layer-2 aggregation + LN + fused L3 matmul
        if max_phase >= 4:
            _edge_phase(
                nc, tc, c, meta, lay=2, Hn=H, Ch=c.C2, ELEM=c.ELEM2,
                hs_full=hs2_full, d_tab=d2t, bln_sb=bln2_sb,
                iota_sb=iota_sb, id_sb=id_sb, idx_sb=idx_sb, dl_sb=dl_sb,
                didx_sb=didx_sb,
                fuse=dict(W_sb=W3a_sb, KC=KC3, Ww=W3w, CO=c.NCLS, Hn2=1,
                          ELEMn=c.ELEM3, hs_loc=hs3_loc, d_next=d3t),
                final=None, y=None)

        if max_phase >= 5 and c.NCORES > 1:
            nc.gpsimd.collective_compute(
                "AllGather", AL.bypass, replica_groups=groups,
                ins=[hs3_loc[:].opt()], outs=[hs3_full[:].opt()])

        # ================= phase F: layer-3 aggregation + LN + log_softmax
        if max_phase >= 6:
            _edge_phase(
                nc, tc, c, meta, lay=3, Hn=1, Ch=c.NCLS, ELEM=c.ELEM3,
                hs_full=hs3_full, d_tab=d3t, bln_sb=bln3_sb,
                iota_sb=iota_sb, id_sb=id_sb, idx_sb=idx_sb, dl_sb=dl_sb,
                didx_sb=didx_sb,
                fuse=None, final=True, y=y)

        cpool_cm.__exit__(None, None, None)
        dram_cm.__exit__(None, None, None)

    nc.compile()
    return nc


def _mm_splits(nc, out_ps, lhs_sb, w_sb, KC, Ww, Plhs):
    """out_ps[:, :Ww] = sum_k lhs_k.T @ W_k, with N split at 512."""
    splits = []
    n0 = 0
    while n0 < Ww:
        nsz = min(512, Ww - n0)
        splits.append((n0, nsz))
        n0 += nsz
    for k in range(KC):
        for (n0, nsz) in splits:
            nc.tensor.matmul(
                out=out_ps[:, n0:n0 + nsz],
                lhsT=lhs_sb[:, k * Plhs:(k + 1) * Plhs],
                rhs=w_sb[:, k * Ww + n0:k * Ww + n0 + nsz],
                start=(k == 0), stop=(k == KC - 1))


def _store_hs(nc, pool, hp, CO, Hn, ELEM, hs_loc, d_tab, t):
    """PSUM [128, CO+2H] -> bf16 hs row tile + bf16 d table row tile."""
    hst = pool.tile([P, ELEM], BF16, tag="hst")
    nc.scalar.copy(hst[:, 0:CO], hp[:, 0:CO])
    nc.vector.tensor_copy(hst[:, CO:CO + Hn], hp[:, CO:CO + Hn])
    if ELEM > CO + Hn:
        nc.vector.memset(hst[:, CO + Hn:ELEM], 0)
    dt = pool.tile([P, P], BF16, tag="dt")
    nc.vector.tensor_copy(dt[:, 0:Hn], hp[:, CO + Hn:CO + 2 * Hn])
    nc.vector.memset(dt[:, Hn:P], 0)
    nc.sync.dma_start(hs_loc[t * P:(t + 1) * P, :], hst[:])
    nc.sync.dma_start(d_tab[t * P:(t + 1) * P, :], dt[:])


def _edge_phase(nc, tc, c: Cfg, meta: Meta, lay, Hn, Ch, ELEM, hs_full, d_tab,
                bln_sb, iota_sb, id_sb, idx_sb, dl_sb, didx_sb,
                fuse, final, y):
    CO = Hn * Ch
    max_nch = max(max(r) for r in meta.nch)
    max_ntot = max(r[0] + r[1] for r in meta.nch)
    merge_den = (Hn == 1)

    with (
        tc.tile_pool(name=f"sb{lay}", bufs=2) as sb,
        tc.tile_pool(name=f"sc{lay}", bufs=4) as sbc,
        tc.tile_pool(name=f"g{lay}", bufs=4) as gp,
        tc.tile_pool(name=f"ps{lay}", bufs=1, space="PSUM") as ps1,
        tc.tile_pool(name=f"psagg{lay}", bufs=2, space="PSUM") as psA,
    ):
        for t in range(c.T):
            nch0, nch1 = meta.nch[t]
            ntot = nch0 + nch1
            agg = psA.tile([P, CO + (1 if merge_den else 0)], F32, tag="agg")
            if merge_den:
                den_ap = agg[:, CO:CO + 1]
            else:
                den_t = ps1.tile([P, Hn], F32, tag="den")
                den_ap = den_t[:]
            if ntot == 0:
                _zero_psum(nc, agg)
                if not merge_den:
                    _zero_psum(nc, den_t)
                _epilogue(nc, sb, ps1, c, meta, lay, t, agg, den_ap, Hn, Ch, CO,
                          bln_sb, id_sb, fuse, final, y)
                continue

            # per-edge d rows for the tile (256B rows from the d table);
            # split per half to stay under the 64-descriptor packet limit
            Dg = gp.tile([P, max_ntot * P], BF16, tag="Dg")
            for hf, nch in ((0, nch0), (1, nch1)):
                if nch == 0:
                    continue
                b0 = 0 if hf == 0 else nch0
                sdo = meta.sd[t] + b0 * (P // 16)
                nc.gpsimd.dma_gather(
                    out_ap=Dg[:, b0 * P:(b0 + nch) * P].rearrange(
                        "p (k d) -> p k d", d=P),
                    in_ap=d_tab[:],
                    idxs_ap=didx_sb[:, sdo:sdo + nch * (P // 16)],
                    num_idxs=nch * P, num_idxs_reg=nch * P, elem_size=P)

            Gs = []
            for hf, nch in ((0, nch0), (1, nch1)):
                if nch == 0:
                    Gs.append(None)
                    continue
                G = gp.tile([P, max_nch * ELEM], BF16, tag="G")
                si = meta.si[t][hf]
                nidx = nch * P
                nc.gpsimd.dma_gather(
                    out_ap=G[:, 0:nch * ELEM].rearrange(
                        "p (k d) -> p k d", d=ELEM),
                    in_ap=hs_full[hf * c.HALF:(hf + 1) * c.HALF, :],
                    idxs_ap=idx_sb[:, si:si + nch * (P // 16)],
                    num_idxs=nidx, num_idxs_reg=nidx, elem_size=ELEM)
                Gs.append(G)

            # ---- batched per-tile prep: eq_all, tsd/leaky/exp for all chunks
            eqa = sbc.tile([P, max_ntot * P], BF16, tag="eqa")
            c0 = meta.sc[t][0]
            dlv = dl_sb[:, c0:c0 + ntot].to_broadcast([P, ntot, P])
            io = iota_sb[:]
            iob = bass.AP(io.tensor, io.offset,
                          [list(io.ap[0]), [0, ntot], list(io.ap[1])])
            nc.vector.tensor_tensor(
                out=eqa[:, 0:ntot * P].rearrange("p (k d) -> p k d", d=P),
                in0=dlv, in1=iob, op=AL.is_equal)
            tsda = sbc.tile([P, max_ntot * Hn], F32, tag="tsda")
            for hf, nch in ((0, nch0), (1, nch1)):
                if nch == 0:
                    continue
                b0 = 0 if hf == 0 else nch0
                Gv = Gs[hf][:, 0:nch * ELEM].rearrange(
                    "p (k d) -> p k d", d=ELEM)[:, :, CO:CO + Hn]
                Dv = Dg[:, b0 * P:(b0 + nch) * P].rearrange(
                    "p (k d) -> p k d", d=P)[:, :, 0:Hn]
                nc.vector.tensor_tensor(
                    out=tsda[:, b0 * Hn:(b0 + nch) * Hn].rearrange(
                        "p (k h) -> p k h", h=Hn),
                    in0=Gv, in1=Dv, op=AL.add)
            lra = sbc.tile([P, max_ntot * Hn], F32, tag="lra")
            nc.vector.scalar_tensor_tensor(
                out=lra[:, 0:ntot * Hn], in0=tsda[:, 0:ntot * Hn],
                scalar=NEG_SLOPE_ATT, in1=tsda[:, 0:ntot * Hn],
                op0=AL.mult, op1=AL.max)
            wfa = sbc.tile([P, max_ntot * Hn], F32, tag="wfa")
            nc.scalar.activation(wfa[:, 0:ntot * Hn], lra[:, 0:ntot * Hn],
                                 AF.Exp)
            wfb = sbc.tile([P, max_ntot * Hn], BF16, tag="wfb")
            nc.vector.tensor_copy(wfb[:, 0:ntot * Hn], wfa[:, 0:ntot * Hn])

            first = True
            gchunk = 0
            for hf, nch in ((0, nch0), (1, nch1)):
                G = Gs[hf]
                for b in range(nch):
                    last = (gchunk == ntot - 1)
                    wf = wfa[:, gchunk * Hn:(gchunk + 1) * Hn]
                    wb = wfb[:, gchunk * Hn:(gchunk + 1) * Hn]
                    eq = eqa[:, gchunk * P:(gchunk + 1) * P]
                    # S[h] = eq * wf[:, h]  (one broadcast op for all heads)
                    S = sbc.tile([P, Hn * P], BF16, tag="S")
                    eq_b = bass.AP(eq.tensor, eq.offset,
                                   [list(eq.ap[0]), [0, Hn], list(eq.ap[1])])
                    nc.vector.tensor_tensor(
                        out=S[:].rearrange("p (h d) -> p h d", h=Hn),
                        in0=eq_b, in1=wf.to_broadcast([P, Hn, P]),
                        op=AL.mult)
                    # psum start/stop zero whole 2KB banks: flag only the
                    # first/last matmul touching each bank of the agg tile.
                    BK = 512  # f32 elems per bank
                    for h in range(Hn):
                        h_first = (h * Ch) % BK == 0
                        h_last = ((h + 1) * Ch) % BK == 0 or (
                            h == Hn - 1 and not merge_den)
                        nc.tensor.matmul(
                            out=agg[:, h * Ch:(h + 1) * Ch],
                            lhsT=S[:, h * P:(h + 1) * P],
                            rhs=G[:, b * ELEM + h * Ch:b * ELEM + (h + 1) * Ch],
                            start=first and h_first, stop=last and h_last)
                    if merge_den:
                        # den shares the agg bank: never start, stop on last
                        nc.tensor.matmul(out=den_ap, lhsT=eq,
                                         rhs=wb[:, 0:1], start=False, stop=last)
                    else:
                        nc.tensor.matmul(out=den_ap, lhsT=eq, rhs=wb,
                                         start=first, stop=last)
                    first = False
                    gchunk += 1

            _epilogue(nc, sb, ps1, c, meta, lay, t, agg, den_ap, Hn, Ch, CO,
                      bln_sb, id_sb, fuse, final, y)


def _zero_psum(nc, ap_tile):
    nc.vector.memset(ap_tile[:], 0)


def _epilogue(nc, sb, ps1, c, meta, lay, t, agg, den_ap, Hn, Ch, CO,
              bln_sb, id_sb, fuse, final, y):
    # out = agg / (den + 1e-16) per head; + bias; LN; (leaky + next matmul) | logsoftmax
    denr = sb.tile([P, Hn], F32, tag="denr")
    nc.vector.tensor_scalar(out=denr[:], in0=den_ap, scalar1=1e-16,
                            scalar2=None, op0=AL.add)
    rec = sb.tile([P, Hn], F32, tag="rec")
    nc.vector.reciprocal(rec[:], denr[:])
    o = sb.tile([P, CO], F32, tag="o")
    for h in range(Hn):
        nc.vector.tensor_scalar(
            out=o[:, h * Ch:(h + 1) * Ch], in0=agg[:, h * Ch:(h + 1) * Ch],
            scalar1=rec[:, h:h + 1], scalar2=None, op0=AL.mult)
    ob = sb.tile([P, CO], F32, tag="ob")
    nc.vector.tensor_tensor(out=ob[:], in0=o[:], in1=bln_sb[:, 0:CO], op=AL.add)
    # LayerNorm
    rs = sb.tile([P, 1], F32, tag="rs")
    nc.vector.tensor_reduce(out=rs[:], in_=ob[:], axis=mybir.AxisListType.X,
                            op=AL.add)
    nm = sb.tile([P, 1], F32, tag="nm")
    nc.vector.tensor_scalar(out=nm[:], in0=rs[:], scalar1=-1.0 / CO,
                            scalar2=None, op0=AL.mult)
    xc = sb.tile([P, CO], F32, tag="xc")
    nc.vector.tensor_scalar(out=xc[:], in0=ob[:], scalar1=nm[:, 0:1],
                            scalar2=None, op0=AL.add)
    sq = sb.tile([P, CO], F32, tag="sq")
    vs = sb.tile([P, 1], F32, tag="vs")
    nc.scalar.activation(sq[:], xc[:], AF.Square, accum_out=vs[:])
    vstd = sb.tile([P, 1], F32, tag="vstd")
    nc.vector.tensor_scalar(out=vstd[:], in0=vs[:], scalar1=1.0 / CO,
                            scalar2=LN_EPS, op0=AL.mult, op1=AL.add)
    sd = sb.tile([P, 1], F32, tag="sd")
    nc.scalar.activation(sd[:], vstd[:], AF.Sqrt)
    rstd = sb.tile([P, 1], F32, tag="rstd")
    nc.vector.reciprocal(rstd[:], sd[:])
    y1 = sb.tile([P, CO], F32, tag="y1")
    nc.vector.scalar_tensor_tensor(
        out=y1[:], in0=xc[:], scalar=rstd[:, 0:1],
        in1=bln_sb[:, CO:2 * CO], op0=AL.mult, op1=AL.mult)
    y2 = sb.tile([P, CO], F32, tag="y2")
    nc.vector.tensor_tensor(out=y2[:], in0=y1[:], in1=bln_sb[:, 2 * CO:3 * CO],
                            op=AL.add)

    if final:
        # log_softmax over CO, write y
        mx = sb.tile([P, 1], F32, tag="mx")
        nc.vector.tensor_reduce(out=mx[:], in_=y2[:],
                                axis=mybir.AxisListType.X, op=AL.max)
        nmx = sb.tile([P, 1], F32, tag="nmx")
        nc.vector.tensor_scalar(out=nmx[:], in0=mx[:], scalar1=-1.0,
                                scalar2=None, op0=AL.mult)
        xs = sb.tile([P, CO], F32, tag="xs")
        nc.vector.tensor_scalar(out=xs[:], in0=y2[:], scalar1=nmx[:, 0:1],
                                scalar2=None, op0=AL.add)
        ex = sb.tile([P, CO], F32, tag="ex")
        se = sb.tile([P, 1], F32, tag="se")
        nc.scalar.activation(ex[:], xs[:], AF.Exp, accum_out=se[:])
        lse = sb.tile([P, 1], F32, tag="lse")
        nc.scalar.activation(lse[:], se[:], AF.Ln)
        nlse = sb.tile([P, 1], F32, tag="nlse")
        nc.vector.tensor_scalar(out=nlse[:], in0=lse[:], scalar1=-1.0,
                                scalar2=None, op0=AL.mult)
        yo = sb.tile([P, CO], F32, tag="yo")
        nc.vector.tensor_scalar(out=yo[:], in0=xs[:], scalar1=nlse[:, 0:1],
                                scalar2=None, op0=AL.add)
        nc.sync.dma_start(y[t * P:(t + 1) * P, :], yo[:])
        return

    # leaky(0.01) -> bf16 x_next; fused next-layer matmul
    x2 = sb.tile([P, CO], BF16, tag="x2")
    nc.vector.scalar_tensor_tensor(
        out=x2[:], in0=y2[:], scalar=NEG_SLOPE_ACT, in1=y2[:],
        op0=AL.mult, op1=AL.max)
    W_sb, KC, Ww = fuse["W_sb"], fuse["KC"], fuse["Ww"]
    CO2, Hn2, ELEMn = fuse["CO"], fuse["Hn2"], fuse["ELEMn"]
    xt2 = sb.tile([P, KC * P], BF16, tag="xt2")
    for k in range(KC):
        scr = ps1.tile([P, P], BF16, tag="scr")
        nc.tensor.transpose(out=scr[:], in_=x2[:, k * P:(k + 1) * P],
                            identity=id_sb[:])
        nc.scalar.copy(xt2[:, k * P:(k + 1) * P], scr[:])
    hp = ps1.tile([P, Ww], F32, tag="hnext")
    _mm_splits(nc, hp, xt2, W_sb, KC, Ww, P)
    _store_hs(nc, sb, hp, CO2, Hn2, ELEMn, fuse["hs_loc"], fuse["d_next"], t)


# --------------------------------------------------------------------------
# entry point
# --------------------------------------------------------------------------

_CACHE = {}


def _get_nc(cfg, meta):
    key = (tuple(sorted(cfg.__dict__.items())),
           tuple(tuple(r) for r in meta.nch))
    if key not in _CACHE:
        _CACHE[key] = build_nc(cfg, meta)
    return _CACHE[key]


def kernel(**inputs):
    inputs = {k: np.asarray(v) for k, v in inputs.items()}
    x = inputs["x"]
    cfg = Cfg(N=x.shape[0], E=inputs["edge_src"].shape[0], F_IN=x.shape[1],
              HEADS=inputs["a_src1"].shape[0], C1=inputs["a_src1"].shape[1],
              C2=inputs["a_src2"].shape[1], NCLS=inputs["W3"].shape[0],
              NCORES=8)
    in_maps, meta = host_prep(cfg, **inputs)
    nc = _get_nc(cfg, meta)
    trace = bool(int(os.environ.get("GAT_TRACE", "0")))
    res = run_bass_kernel_spmd(nc, in_maps, core_ids=list(range(cfg.NCORES)),
                               trace=trace)
    global LAST_EXEC_NS
    LAST_EXEC_NS = res.exec_time_ns
    out = np.concatenate(
        [res.results[cc]["y"][:cfg.NL] for cc in range(cfg.NCORES)], axis=0)
    return out.astype(np.float32)


LAST_EXEC_NS = None


if __name__ == "__main__":
    pass



# revision 28
# speedup vs baseline: 1.0252x; 1.0252x over previous
"""3-layer GAT on 8 Trainium2 NeuronCores (Bass/Tile).

Strategy: partition nodes across the 8 cores (graph parallel); edges live with
their destination core so segment-softmax/aggregation stay local; per layer,
all-gather the (bf16) node features + attention source logits in CHUNKS that
overlap the producing phase; gather source rows per edge chunk with
dma_gather; aggregate with selection-matrix matmuls on the PE.  Per-edge
destination logits are computed on the PE from a transposed one-hot matrix
(no d-gather).

Self-contained: only imports the system concourse install.
"""

import os
import sys

for _p in ("/opt/trn_rl_repo", "/root/.axon_site/_ro/trn_rl_repo"):
    if os.path.isdir(_p) and _p not in sys.path:
        sys.path.insert(0, _p)

from dataclasses import dataclass

import ml_dtypes
import numpy as np

import concourse.bacc as bacc
import concourse.bass as bass
import concourse.tile as tile
from concourse import mybir
from concourse.bass_utils import run_bass_kernel_spmd

P = 128
BF16 = mybir.dt.bfloat16
F32 = mybir.dt.float32
I16 = mybir.dt.int16
AL = mybir.AluOpType
AF = mybir.ActivationFunctionType

NEG_SLOPE_ATT = 0.2
NEG_SLOPE_ACT = 0.01
LN_EPS = 1e-5


def _ceil(a, b):
    return -(-a // b)


def _pad_elem(n_f32_elems):
    """bf16 row length (elements) padded so row bytes are a multiple of 256."""
    return _ceil(n_f32_elems * 2, 256) * 128


@dataclass
class Cfg:
    N: int = 50000
    E: int = 400000
    F_IN: int = 256
    HEADS: int = 4
    C1: int = 256
    C2: int = 128
    NCLS: int = 32
    NCORES: int = 8

    def __post_init__(self):
        assert self.N % self.NCORES == 0
        self.NL = self.N // self.NCORES
        self.T = _ceil(self.NL, P)
        self.NLP = self.T * P
        self.NPTOT = self.NLP * self.NCORES
        # the full hs table is split into two HALF tensors (half-major
        # layout), each written by a single AllGather so the collectives
        # pipeline with the producing/consuming phases; int16 gather
        # indices are relative to the owning half.
        q = self.T // 2
        self.CHT = [self.T - q, q]
        self.CH_T0 = np.cumsum([0] + self.CHT).tolist()  # half start tile
        self.CH_ROWS = [c * P for c in self.CHT]
        self.CH_BASE = np.cumsum(
            [0] + [r * self.NCORES for r in self.CH_ROWS]).tolist()
        self.B = self.CH_BASE[1]
        assert self.B <= 32767 and self.NPTOT - self.B <= 32767
        H = self.HEADS
        self.CO1 = H * self.C1
        self.CO2 = H * self.C2
        assert self.F_IN % P == 0 and self.CO1 % P == 0 and self.CO2 % P == 0
        self.ELEM1 = _pad_elem(self.CO1 + H)
        self.ELEM2 = _pad_elem(self.CO2 + H)
        self.ELEM3 = _pad_elem(self.NCLS + 1)


@dataclass
class Meta:
    nch: list  # [T][2] chunk counts (common across cores)
    si: list   # [T][2] idx16 column offsets
    sc: list   # [T][2] dstloc column offsets
    SI: int
    SC: int


def host_prep(cfg: Cfg, x, edge_src, edge_dst,
              W1, a_src1, a_dst1, b1, ln1_g, ln1_b,
              W2, a_src2, a_dst2, b2, ln2_g, ln2_b,
              W3, a_src3, a_dst3, b3, ln3_g, ln3_b):
    """Build per-core input maps + the (common) chunk structure."""
    c = cfg
    bf = ml_dtypes.bfloat16

    # ---- append self loops, shard edges by destination core
    loops = np.arange(c.N, dtype=np.int64)
    src = np.concatenate([edge_src.astype(np.int64), loops])
    dst = np.concatenate([edge_dst.astype(np.int64), loops])

    dst_core = dst // c.NL
    dstloc = dst - dst_core * c.NL
    tile_id = dstloc // P

    # padded-global source row in CHUNK-MAJOR layout:
    # row(cc,t,rr) = CH_BASE[k] + cc*CH_ROWS[k] + (t-CH_T0[k])*P + rr
    s_core = src // c.NL
    s_loc = src % c.NL
    s_t = s_loc // P
    s_rr = s_loc - s_t * P
    chunk_of_tile = np.zeros(c.T, np.int64)
    for k in range(2):
        chunk_of_tile[c.CH_T0[k]:c.CH_T0[k + 1]] = k
    half = chunk_of_tile[s_t]
    ch_rows = np.asarray(c.CH_ROWS, np.int64)
    ch_t0 = np.asarray(c.CH_T0[:2], np.int64)
    idx16 = (s_core * ch_rows[half] + (s_t - ch_t0[half]) * P + s_rr)

    # group edges per (core, tile, half)
    counts = np.zeros((c.NCORES, c.T, 2), np.int64)
    np.add.at(counts, (dst_core, tile_id, half), 1)
    nch = _ceil_arr(counts.max(axis=0), P)  # [T,2] chunks
    si = np.zeros((c.T, 2), np.int64)
    sc = np.zeros((c.T, 2), np.int64)
    acc_si = acc_sc = 0
    for t in range(c.T):
        for h in range(2):
            si[t, h] = acc_si
            sc[t, h] = acc_sc
            acc_si += int(nch[t, h]) * (P // 16)
            acc_sc += int(nch[t, h])
    SI, SC = int(acc_si), int(acc_sc)
    meta = Meta(nch=nch.tolist(), si=si.tolist(), sc=sc.tolist(), SI=SI, SC=SC)

    # ---- per-core index / dstloc tables
    order = np.lexsort((half, tile_id, dst_core))
    src_s, half_s, t_s, core_s = (idx16[order], half[order], tile_id[order],
                                  dst_core[order])
    dstrel_s = (dstloc - tile_id * P)[order]

    starts = np.zeros((c.NCORES, c.T, 2), np.int64)
    run = 0
    for cc in range(c.NCORES):
        for t in range(c.T):
            for h in range(2):
                starts[cc, t, h] = run
                run += int(counts[cc, t, h])

    idx_tabs, eq_tabs, eqt_tabs = [], [], []
    jj = np.arange(P, dtype=np.int64)
    for cc in range(c.NCORES):
        itab = np.zeros((16, SI), np.int16)
        eqtab = np.zeros((P, SC * P), bf)    # [e, (k, j)] one-hot by dstrel
        eqttab = np.zeros((P, SC * P), bf)   # [j, (k, e)] transposed one-hot
        for t in range(c.T):
            for h in range(2):
                m = int(counts[cc, t, h])
                n = int(nch[t, h])
                if n == 0:
                    continue
                s0 = int(starts[cc, t, h])
                iv = np.zeros(n * P, np.int16)
                iv[:m] = src_s[s0:s0 + m].astype(np.int16)
                cols = int(si[t, h])
                blk = iv.reshape(n * P // 16, 16).T  # idx k -> [k%16, k//16]
                itab[:, cols:cols + n * (P // 16)] = blk
                dv = np.full(n * P, -1, np.int64)
                dv[:m] = dstrel_s[s0:s0 + m]
                dvk = dv.reshape(n, P)                        # [k, e]
                # eq[e, (k, j)] = (dstrel(k, e) == j)
                eq_blk = (dvk[:, :, None] == jj[None, None, :])  # [k, e, j]
                c0 = int(sc[t, h])
                eqtab[:, c0 * P:(c0 + n) * P] = np.ascontiguousarray(
                    eq_blk.transpose(1, 0, 2).reshape(P, n * P)).astype(bf)
                # eqT[j, (k, e)] = (dstrel(k, e) == j)
                eqttab[:, c0 * P:(c0 + n) * P] = np.ascontiguousarray(
                    eq_blk.transpose(2, 0, 1).reshape(P, n * P)).astype(bf)
        idx_tabs.append(np.tile(itab, (8, 1)))
        eq_tabs.append(eqtab)
        eqt_tabs.append(eqttab)

    # ---- weights (augmented with U = W.T @ a columns), bf16
    def aug(W, a_s, a_d, H, C):
        WT = W.T.astype(np.float64)                      # [Fin, H*C]
        U_s = np.zeros((WT.shape[0], H))
        U_d = np.zeros((WT.shape[0], H))
        for h in range(H):
            U_s[:, h] = WT[:, h * C:(h + 1) * C] @ a_s[h].astype(np.float64)
            U_d[:, h] = WT[:, h * C:(h + 1) * C] @ a_d[h].astype(np.float64)
        return np.concatenate([WT, U_s, U_d], axis=1).astype(bf)

    W1a = aug(W1, a_src1, a_dst1, c.HEADS, c.C1)   # [F_IN, CO1+2H]
    W2a = aug(W2, a_src2, a_dst2, c.HEADS, c.C2)   # [CO1, CO2+2H]
    W3a = aug(W3, a_src3, a_dst3, 1, c.NCLS)       # [CO2, NCLS+2]

    def bln(b, g, be, D):
        row = np.concatenate([b, g, be]).astype(np.float32)[None, :]
        return np.repeat(row, P, axis=0)           # [128, 3D]

    bln1 = bln(b1, ln1_g, ln1_b, c.CO1)
    bln2 = bln(b2, ln2_g, ln2_b, c.CO2)
    bln3 = bln(b3, ln3_g, ln3_b, c.NCLS)

    ident = np.eye(P, dtype=bf)

    in_maps = []
    for cc in range(c.NCORES):
        xl = np.zeros((c.NLP, c.F_IN), np.float32)
        xl[:c.NL] = x[cc * c.NL:(cc + 1) * c.NL]
        in_maps.append({
            "xT": np.ascontiguousarray(xl.T).astype(bf),
            "W1a": W1a, "W2a": W2a, "W3a": W3a,
            "bln1": bln1, "bln2": bln2, "bln3": bln3,
            "idx16": idx_tabs[cc], "eqtab": eq_tabs[cc],
            "eqttab": eqt_tabs[cc], "ident": ident,
        })
    return in_maps, meta


def _ceil_arr(a, b):
    return -(-a // b)


# --------------------------------------------------------------------------
# device program
# --------------------------------------------------------------------------

def build_nc(cfg: Cfg, meta: Meta, max_phase: int = 6):
    c = cfg
    H = c.HEADS
    nc = bacc.Bacc("TRN2", target_bir_lowering=False, debug=False,
                   num_devices=c.NCORES, enable_partition_id=False)

    # ---- I/O
    xT = nc.dram_tensor("xT", [c.F_IN, c.NLP], BF16, kind="ExternalInput").ap()
    W1a = nc.dram_tensor("W1a", [c.F_IN, c.CO1 + 2 * H], BF16, kind="ExternalInput").ap()
    W2a = nc.dram_tensor("W2a", [c.CO1, c.CO2 + 2 * H], BF16, kind="ExternalInput").ap()
    W3a = nc.dram_tensor("W3a", [c.CO2, c.NCLS + 2], BF16, kind="ExternalInput").ap()
    bln1 = nc.dram_tensor("bln1", [P, 3 * c.CO1], F32, kind="ExternalInput").ap()
    bln2 = nc.dram_tensor("bln2", [P, 3 * c.CO2], F32, kind="ExternalInput").ap()
    bln3 = nc.dram_tensor("bln3", [P, 3 * c.NCLS], F32, kind="ExternalInput").ap()
    idx16 = nc.dram_tensor("idx16", [P, meta.SI], I16, kind="ExternalInput").ap()
    eqtab = nc.dram_tensor("eqtab", [P, meta.SC * P], BF16,
                           kind="ExternalInput").ap()
    eqttab = nc.dram_tensor("eqttab", [P, meta.SC * P], BF16,
                            kind="ExternalInput").ap()
    ident = nc.dram_tensor("ident", [P, P], BF16, kind="ExternalInput").ap()
    y = nc.dram_tensor("y", [c.NLP, c.NCLS], F32, kind="ExternalOutput").ap()

    groups = [list(range(c.NCORES))]

    def ag_half(nc, hs_loc, hs_halves, k):
        """AllGather one half table (single writer per Shared tensor)."""
        r0 = c.CH_T0[k] * P
        rows = c.CH_ROWS[k]
        nc.gpsimd.collective_compute(
            "AllGather", AL.bypass, replica_groups=groups,
            ins=[hs_loc[r0:r0 + rows, :].opt()],
            outs=[hs_halves[k][:, :].opt()])

    with tile.TileContext(nc) as tc:
        # ---- persistent DRAM intermediates
        dram_cm = tc.tile_pool(name="dram", bufs=1, space="DRAM")
        dram = dram_cm.__enter__()
        aspace = "Shared" if c.NCORES > 4 else "Local"
        R0, R1 = c.B, c.NPTOT - c.B
        hs1_loc = dram.tile([c.NLP, c.ELEM1], BF16)
        hs1_h0 = dram.tile([R0, c.ELEM1], BF16, addr_space=aspace)
        hs1_h1 = dram.tile([R1, c.ELEM1], BF16, addr_space=aspace)
        hs1_h = [hs1_h0, hs1_h1]
        hs2_loc = dram.tile([c.NLP, c.ELEM2], BF16)
        hs2_h0 = dram.tile([R0, c.ELEM2], BF16, addr_space=aspace)
        hs2_h1 = dram.tile([R1, c.ELEM2], BF16, addr_space=aspace)
        hs2_h = [hs2_h0, hs2_h1]
        hs3_loc = dram.tile([c.NLP, c.ELEM3], BF16)
        hs3_h0 = dram.tile([R0, c.ELEM3], BF16, addr_space=aspace)
        hs3_h1 = dram.tile([R1, c.ELEM3], BF16, addr_space=aspace)
        hs3_h = [hs3_h0, hs3_h1]

        # ---- persistent SBUF constants
        cpool_cm = tc.tile_pool(name="const", bufs=1)
        cpool = cpool_cm.__enter__()
        KC1 = c.F_IN // P
        W1w = c.CO1 + 2 * H
        W1a_sb = cpool.tile([P, KC1 * W1w], BF16)
        for k in range(KC1):
            nc.sync.dma_start(W1a_sb[:, k * W1w:(k + 1) * W1w],
                              W1a[k * P:(k + 1) * P, :])
        KC2 = c.CO1 // P
        W2w = c.CO2 + 2 * H
        W2a_sb = cpool.tile([P, KC2 * W2w], BF16)
        for k in range(KC2):
            nc.sync.dma_start(W2a_sb[:, k * W2w:(k + 1) * W2w],
                              W2a[k * P:(k + 1) * P, :])
        KC3 = c.CO2 // P
        W3w = c.NCLS + 2
        W3a_sb = cpool.tile([P, KC3 * W3w], BF16)
        for k in range(KC3):
            nc.sync.dma_start(W3a_sb[:, k * W3w:(k + 1) * W3w],
                              W3a[k * P:(k + 1) * P, :])
        bln1_sb = cpool.tile([P, 3 * c.CO1], F32)
        nc.sync.dma_start(bln1_sb[:], bln1[:])
        bln2_sb = cpool.tile([P, 3 * c.CO2], F32)
        nc.sync.dma_start(bln2_sb[:], bln2[:])
        bln3_sb = cpool.tile([P, 3 * c.NCLS], F32)
        nc.sync.dma_start(bln3_sb[:], bln3[:])
        idx_sb = cpool.tile([P, meta.SI], I16)
        nc.sync.dma_start(idx_sb[:], idx16[:])
        id_sb = cpool.tile([P, P], BF16)
        nc.sync.dma_start(id_sb[:], ident[:])
        # per-tile destination attention logits (bf16), layer 1..3
        d1_sb = cpool.tile([P, c.T * H], BF16)
        d2_sb = cpool.tile([P, c.T * H], BF16)
        d3_sb = cpool.tile([P, c.T * 1], BF16)

        # ================= phase A: h1 = x @ W1a (per local node tile)
        with (
            tc.tile_pool(name="pA", bufs=3) as pA,
            tc.tile_pool(name="pAp", bufs=2, space="PSUM") as pAp,
        ):
            for t in range(c.T):
                xt = pA.tile([P, KC1 * P], BF16, tag="xt")
                for k in range(KC1):
                    nc.sync.dma_start(xt[:, k * P:(k + 1) * P],
                                      xT[k * P:(k + 1) * P, t * P:(t + 1) * P])
                hp = pAp.tile([P, W1w], F32, tag="hp")
                _mm_splits(nc, hp, xt, W1a_sb, KC1, W1w, P)
                _store_hs(nc, pA, hp, c.CO1, H, c.ELEM1, hs1_loc, d1_sb, t)

        if max_phase >= 1:
            ag_half(nc, hs1_loc, hs1_h, 0)
            ag_half(nc, hs1_loc, hs1_h, 1)

        # ================= layer-1 aggregation + LN + fused L2 matmul
        if max_phase >= 2:
            _edge_phase(
                nc, tc, c, meta, lay=1, Hn=H, Ch=c.C1, ELEM=c.ELEM1,
                hs_h=hs1_h, d_sb=d1_sb, bln_sb=bln1_sb,
                id_sb=id_sb, idx_sb=idx_sb, eqtab=eqtab, eqttab=eqttab,
                fuse=dict(W_sb=W2a_sb, KC=KC2, Ww=W2w, CO=c.CO2, Hn2=H,
                          ELEMn=c.ELEM2, hs_loc=hs2_loc, d_next=d2_sb,
                          hs_h_n=hs2_h, ag=ag_half if max_phase >= 3 else None),
                final=None, y=None)

        # ================= layer-2 aggregation + LN + fused L3 matmul
        if max_phase >= 4:
            _edge_phase(
                nc, tc, c, meta, lay=2, Hn=H, Ch=c.C2, ELEM=c.ELEM2,
                hs_h=hs2_h, d_sb=d2_sb, bln_sb=bln2_sb,
                id_sb=id_sb, idx_sb=idx_sb, eqtab=eqtab, eqttab=eqttab,
                fuse=dict(W_sb=W3a_sb, KC=KC3, Ww=W3w, CO=c.NCLS, Hn2=1,
                          ELEMn=c.ELEM3, hs_loc=hs3_loc, d_next=d3_sb,
                          hs_h_n=hs3_h, ag=ag_half if max_phase >= 5 else None),
                final=None, y=None)

        # ================= layer-3 aggregation + LN + log_softmax
        if max_phase >= 6:
            _edge_phase(
                nc, tc, c, meta, lay=3, Hn=1, Ch=c.NCLS, ELEM=c.ELEM3,
                hs_h=hs3_h, d_sb=d3_sb, bln_sb=bln3_sb,
                id_sb=id_sb, idx_sb=idx_sb, eqtab=eqtab, eqttab=eqttab,
                fuse=None, final=True, y=y)

        cpool_cm.__exit__(None, None, None)
        dram_cm.__exit__(None, None, None)

    nc.compile()
    return nc


def _mm_splits(nc, out_ps, lhs_sb, w_sb, KC, Ww, Plhs):
    """out_ps[:, :Ww] = sum_k lhs_k.T @ W_k, with N split at 512."""
    splits = []
    n0 = 0
    while n0 < Ww:
        nsz = min(512, Ww - n0)
        splits.append((n0, nsz))
        n0 += nsz
    for k in range(KC):
        for (n0, nsz) in splits:
            nc.tensor.matmul(
                out=out_ps[:, n0:n0 + nsz],
                lhsT=lhs_sb[:, k * Plhs:(k + 1) * Plhs],
                rhs=w_sb[:, k * Ww + n0:k * Ww + n0 + nsz],
                start=(k == 0), stop=(k == KC - 1))


def _store_hs(nc, pool, hp, CO, Hn, ELEM, hs_loc, d_sb, t):
    """PSUM [128, CO+2H] -> bf16 hs row tile; d column -> persistent SBUF."""
    hst = pool.tile([P, ELEM], BF16, tag="hst")
    nc.scalar.copy(hst[:, 0:CO + Hn], hp[:, 0:CO + Hn])
    nc.vector.tensor_copy(d_sb[:, t * Hn:(t + 1) * Hn],
                          hp[:, CO + Hn:CO + 2 * Hn])
    nc.sync.dma_start(hs_loc[t * P:(t + 1) * P, :], hst[:])


def _bcast3(ap, n_mid):
    """[P, X] AP -> [P, n_mid, X] with stride-0 middle dim."""
    return bass.AP(ap.tensor, ap.offset,
                   [list(ap.ap[0]), [0, n_mid], list(ap.ap[1])])


def _edge_phase(nc, tc, c: Cfg, meta: Meta, lay, Hn, Ch, ELEM, hs_h, d_sb,
                bln_sb, id_sb, idx_sb, eqtab, eqttab,
                fuse, final, y):
    CO = Hn * Ch
    max_nch = max(max(r) for r in meta.nch)
    max_ntot = max(r[0] + r[1] for r in meta.nch)
    merge_den = (Hn == 1)
    # next-layer half boundaries: tile index -> half id to all-gather
    ag_after = {c.CH_T0[k + 1] - 1: k for k in range(2)} if (
        fuse and fuse.get("ag")) else {}

    with (
        tc.tile_pool(name=f"sb{lay}", bufs=2) as sb,
        tc.tile_pool(name=f"sc{lay}", bufs=4) as sbc,
        tc.tile_pool(name=f"sq{lay}", bufs=2) as sbq,
        tc.tile_pool(name=f"g{lay}", bufs=4) as gp,
        tc.tile_pool(name=f"ps{lay}", bufs=1, space="PSUM") as ps1,
        tc.tile_pool(name=f"psagg{lay}", bufs=2, space="PSUM") as psA,
    ):
        for t in range(c.T):
            nch0, nch1 = meta.nch[t]
            ntot = nch0 + nch1
            agg = psA.tile([P, CO + (1 if merge_den else 0)], F32, tag="agg")
            # dd: [dvals(ntot*Hn) | den(Hn)] in one PSUM bank
            dd = ps1.tile([P, (max_ntot + 1) * Hn], F32, tag="dd")
            den_ap = agg[:, CO:CO + 1] if merge_den else dd[:, ntot * Hn:
                                                            (ntot + 1) * Hn]

            c0 = meta.sc[t][0]
            eqa = sbq.tile([P, max_ntot * P], BF16, tag="eqa")
            nc.sync.dma_start(eqa[:, 0:ntot * P],
                              eqtab[:, c0 * P:(c0 + ntot) * P])
            eqT = sbq.tile([P, max_ntot * P], BF16, tag="eqT")
            nc.sync.dma_start(eqT[:, 0:ntot * P],
                              eqttab[:, c0 * P:(c0 + ntot) * P])

            Gs = []
            for hf, nch in ((0, nch0), (1, nch1)):
                if nch == 0:
                    Gs.append(None)
                    continue
                G = gp.tile([P, max_nch * ELEM], BF16, tag="G")
                si = meta.si[t][hf]
                nidx = nch * P
                nc.gpsimd.dma_gather(
                    out_ap=G[:, 0:nch * ELEM].rearrange(
                        "p (k d) -> p k d", d=ELEM),
                    in_ap=hs_h[hf][:, :],
                    idxs_ap=idx_sb[:, si:si + nch * (P // 16)],
                    num_idxs=nidx, num_idxs_reg=nidx, elem_size=ELEM)
                Gs.append(G)

            # ---- per-edge dst logits via PE: dd[:, b*Hn:(b+1)*Hn] = eqT_b^T @ d
            # dvals form a closed accumulation group (PSUM is only readable
            # after the group's stop); den later opens a second group in the
            # same bank — safe because den's rhs (wfb) depends on tsda which
            # consumed the dvals first.
            dt_ap = d_sb[:, t * Hn:(t + 1) * Hn]
            for b in range(ntot):
                nc.tensor.matmul(
                    out=dd[:, b * Hn:(b + 1) * Hn],
                    lhsT=eqT[:, b * P:(b + 1) * P], rhs=dt_ap,
                    start=(b == 0),
                    stop=(b == ntot - 1))

            # ---- tsd = s (from G) + d; leaky; exp
            tsda = sbc.tile([P, max_ntot * Hn], F32, tag="tsda")
            for hf, nch in ((0, nch0), (1, nch1)):
                if nch == 0:
                    continue
                b0 = 0 if hf == 0 else nch0
                Gv = Gs[hf][:, 0:nch * ELEM].rearrange(
                    "p (k d) -> p k d", d=ELEM)[:, :, CO:CO + Hn]
                Dv = dd[:, b0 * Hn:(b0 + nch) * Hn].rearrange(
                    "p (k h) -> p k h", h=Hn)
                nc.vector.tensor_tensor(
                    out=tsda[:, b0 * Hn:(b0 + nch) * Hn].rearrange(
                        "p (k h) -> p k h", h=Hn),
                    in0=Gv, in1=Dv, op=AL.add)
            lra = sbc.tile([P, max_ntot * Hn], F32, tag="lra")
            nc.vector.scalar_tensor_tensor(
                out=lra[:, 0:ntot * Hn], in0=tsda[:, 0:ntot * Hn],
                scalar=NEG_SLOPE_ATT, in1=tsda[:, 0:ntot * Hn],
                op0=AL.mult, op1=AL.max)
            wfa = sbc.tile([P, max_ntot * Hn], F32, tag="wfa")
            nc.scalar.activation(wfa[:, 0:ntot * Hn], lra[:, 0:ntot * Hn],
                                 AF.Exp)
            wfb = sbc.tile([P, max_ntot * Hn], BF16, tag="wfb")
            nc.vector.tensor_copy(wfb[:, 0:ntot * Hn], wfa[:, 0:ntot * Hn])

            # ---- S_h = eq * wf_h  (one batched op per head)
            S = sbq.tile([P, Hn * max_ntot * P], BF16, tag="S")
            wfa_ap = wfa[:, 0:ntot * Hn]
            for h in range(Hn):
                wf_h = bass.AP(wfa_ap.tensor, wfa_ap.offset + h,
                               [list(wfa_ap.ap[0]), [Hn, ntot], [0, P]])
                nc.vector.tensor_tensor(
                    out=S[:, h * ntot * P:(h + 1) * ntot * P].rearrange(
                        "p (k d) -> p k d", d=P),
                    in0=eqa[:, 0:ntot * P].rearrange("p (k d) -> p k d", d=P),
                    in1=wf_h, op=AL.mult)

            # ---- aggregation matmuls
            first = True
            first_den = True
            gchunk = 0
            BK = 512  # f32 elems per psum bank
            for hf, nch in ((0, nch0), (1, nch1)):
                G = Gs[hf]
                for b in range(nch):
                    last = (gchunk == ntot - 1)
                    eq = eqa[:, gchunk * P:(gchunk + 1) * P]
                    for h in range(Hn):
                        h_first = (h * Ch) % BK == 0
                        h_last = ((h + 1) * Ch) % BK == 0 or (
                            h == Hn - 1 and not merge_den)
                        nc.tensor.matmul(
                            out=agg[:, h * Ch:(h + 1) * Ch],
                            lhsT=S[:, h * ntot * P + gchunk * P:
                                   h * ntot * P + (gchunk + 1) * P],
                            rhs=G[:, b * ELEM + h * Ch:b * ELEM + (h + 1) * Ch],
                            start=first and h_first, stop=last and h_last)
                    if merge_den:
                        nc.tensor.matmul(out=den_ap, lhsT=eq,
                                         rhs=wfb[:, gchunk:gchunk + 1],
                                         start=False, stop=last)
                    else:
                        nc.tensor.matmul(
                            out=den_ap, lhsT=eq,
                            rhs=wfb[:, gchunk * Hn:(gchunk + 1) * Hn],
                            start=first_den, stop=last)
                        first_den = False
                    first = False
                    gchunk += 1

            _epilogue(nc, sb, ps1, c, meta, lay, t, agg, den_ap, Hn, Ch, CO,
                      bln_sb, id_sb, fuse, final, y)
            if t in ag_after:
                fuse["ag"](nc, fuse["hs_loc"], fuse["hs_h_n"], ag_after[t])


def _epilogue(nc, sb, ps1, c, meta, lay, t, agg, den_ap, Hn, Ch, CO,
              bln_sb, id_sb, fuse, final, y):
    # out = agg / den per head; + bias; LN; (leaky + next matmul) | logsoftmax
    denr = sb.tile([P, Hn], F32, tag="denr")
    nc.vector.tensor_scalar(out=denr[:], in0=den_ap, scalar1=1e-16,
                            scalar2=None, op0=AL.add)
    rec = sb.tile([P, Hn], F32, tag="rec")
    nc.vector.reciprocal_approx_fast(rec[:], denr[:])
    ob = sb.tile([P, CO], F32, tag="ob")
    if Hn == 1:
        nc.vector.scalar_tensor_tensor(
            out=ob[:], in0=agg[:, 0:CO], scalar=rec[:, 0:1],
            in1=bln_sb[:, 0:CO], op0=AL.mult, op1=AL.add)
    else:
        o = sb.tile([P, CO], F32, tag="o")
        rap = rec[:]
        rec_b = bass.AP(rap.tensor, rap.offset,
                        [list(rap.ap[0]), [1, Hn], [0, Ch]])
        nc.vector.tensor_tensor(
            out=o[:].rearrange("p (h d) -> p h d", h=Hn),
            in0=agg[:, 0:CO].rearrange("p (h d) -> p h d", h=Hn),
            in1=rec_b, op=AL.mult)
        nc.vector.tensor_tensor(out=ob[:], in0=o[:], in1=bln_sb[:, 0:CO],
                                op=AL.add)
    # LayerNorm
    rs = sb.tile([P, 1], F32, tag="rs")
    nc.vector.tensor_reduce(out=rs[:], in_=ob[:], axis=mybir.AxisListType.X,
                            op=AL.add)
    nm = sb.tile([P, 1], F32, tag="nm")
    nc.vector.tensor_scalar(out=nm[:], in0=rs[:], scalar1=-1.0 / CO,
                            scalar2=None, op0=AL.mult)
    xc = sb.tile([P, CO], F32, tag="xc")
    nc.vector.tensor_scalar(out=xc[:], in0=ob[:], scalar1=nm[:, 0:1],
                            scalar2=None, op0=AL.add)
    sq = sb.tile([P, CO], F32, tag="sq")
    vs = sb.tile([P, 1], F32, tag="vs")
    nc.scalar.activation(sq[:], xc[:], AF.Square, accum_out=vs[:])
    vstd = sb.tile([P, 1], F32, tag="vstd")
    nc.vector.tensor_scalar(out=vstd[:], in0=vs[:], scalar1=1.0 / CO,
                            scalar2=LN_EPS, op0=AL.mult, op1=AL.add)
    rv = sb.tile([P, 1], F32, tag="rv")
    nc.vector.reciprocal_approx_fast(rv[:], vstd[:])
    rstd = sb.tile([P, 1], F32, tag="rstd")
    nc.scalar.activation(rstd[:], rv[:], AF.Sqrt)
    y1 = sb.tile([P, CO], F32, tag="y1")
    nc.vector.scalar_tensor_tensor(
        out=y1[:], in0=xc[:], scalar=rstd[:, 0:1],
        in1=bln_sb[:, CO:2 * CO], op0=AL.mult, op1=AL.mult)
    y2 = sb.tile([P, CO], F32, tag="y2")
    nc.vector.tensor_tensor(out=y2[:], in0=y1[:], in1=bln_sb[:, 2 * CO:3 * CO],
                            op=AL.add)

    if final:
        # log_softmax over CO, write y
        mx = sb.tile([P, 1], F32, tag="mx")
        nc.vector.tensor_reduce(out=mx[:], in_=y2[:],
                                axis=mybir.AxisListType.X, op=AL.max)
        nmx = sb.tile([P, 1], F32, tag="nmx")
        nc.vector.tensor_scalar(out=nmx[:], in0=mx[:], scalar1=-1.0,
                                scalar2=None, op0=AL.mult)
        xs = sb.tile([P, CO], F32, tag="xs")
        nc.vector.tensor_scalar(out=xs[:], in0=y2[:], scalar1=nmx[:, 0:1],
                                scalar2=None, op0=AL.add)
        ex = sb.tile([P, CO], F32, tag="ex")
        se = sb.tile([P, 1], F32, tag="se")
        nc.scalar.activation(ex[:], xs[:], AF.Exp, accum_out=se[:])
        lse = sb.tile([P, 1], F32, tag="lse")
        nc.scalar.activation(lse[:], se[:], AF.Ln)
        nlse = sb.tile([P, 1], F32, tag="nlse")
        nc.vector.tensor_scalar(out=nlse[:], in0=lse[:], scalar1=-1.0,
                                scalar2=None, op0=AL.mult)
        yo = sb.tile([P, CO], F32, tag="yo")
        nc.vector.tensor_scalar(out=yo[:], in0=xs[:], scalar1=nlse[:, 0:1],
                                scalar2=None, op0=AL.add)
        nc.sync.dma_start(y[t * P:(t + 1) * P, :], yo[:])
        return

    # leaky(0.01) -> bf16 x_next; fused next-layer matmul
    x2 = sb.tile([P, CO], BF16, tag="x2")
    nc.vector.scalar_tensor_tensor(
        out=x2[:], in0=y2[:], scalar=NEG_SLOPE_ACT, in1=y2[:],
        op0=AL.mult, op1=AL.max)
    W_sb, KC, Ww = fuse["W_sb"], fuse["KC"], fuse["Ww"]
    CO2, Hn2, ELEMn = fuse["CO"], fuse["Hn2"], fuse["ELEMn"]
    xt2 = sb.tile([P, KC * P], BF16, tag="xt2")
    for k in range(KC):
        scr = ps1.tile([P, P], BF16, tag="scr")
        nc.tensor.transpose(out=scr[:], in_=x2[:, k * P:(k + 1) * P],
                            identity=id_sb[:])
        nc.scalar.copy(xt2[:, k * P:(k + 1) * P], scr[:])
    hp = ps1.tile([P, Ww], F32, tag="hnext")
    _mm_splits(nc, hp, xt2, W_sb, KC, Ww, P)
    _store_hs(nc, sb, hp, CO2, Hn2, ELEMn, fuse["hs_loc"], fuse["d_next"], t)


# --------------------------------------------------------------------------
# entry point
# --------------------------------------------------------------------------

_CACHE = {}


def _get_nc(cfg, meta):
    key = (tuple(sorted((k, v) for k, v in cfg.__dict__.items()
                        if isinstance(v, (int, float, str)))),
           tuple(tuple(r) for r in meta.nch))
    if key not in _CACHE:
        _CACHE[key] = build_nc(cfg, meta)
    return _CACHE[key]


def kernel(**inputs):
    inputs = {k: np.asarray(v) for k, v in inputs.items()}
    x = inputs["x"]
    cfg = Cfg(N=x.shape[0], E=inputs["edge_src"].shape[0], F_IN=x.shape[1],
              HEADS=inputs["a_src1"].shape[0], C1=inputs["a_src1"].shape[1],
              C2=inputs["a_src2"].shape[1], NCLS=inputs["W3"].shape[0],
              NCORES=8)
    in_maps, meta = host_prep(cfg, **inputs)
    nc = _get_nc(cfg, meta)
    trace = bool(int(os.environ.get("GAT_TRACE", "0")))
    res = run_bass_kernel_spmd(nc, in_maps, core_ids=list(range(cfg.NCORES)),
                               trace=trace)
    global LAST_EXEC_NS
    LAST_EXEC_NS = res.exec_time_ns
    out = np.concatenate(
        [res.results[cc]["y"][:cfg.NL] for cc in range(cfg.NCORES)], axis=0)
    return out.astype(np.float32)


LAST_EXEC_NS = None


if __name__ == "__main__":
    pass


# revision 31
# speedup vs baseline: 18.0808x; 17.6356x over previous
"""3-layer GAT on 8 Trainium2 NeuronCores (Bass/Tile).

Strategy: partition nodes across the 8 cores (graph parallel); edges live with
their destination core so segment-softmax/aggregation stay local; per layer,
all-gather the (bf16) node features + attention source logits in CHUNKS that
overlap the producing phase; gather source rows per edge chunk with
dma_gather; aggregate with selection-matrix matmuls on the PE.  Per-edge
destination logits are computed on the PE from a transposed one-hot matrix
(no d-gather).

Self-contained: only imports the system concourse install.
"""

import os
import sys

for _p in ("/opt/trn_rl_repo", "/root/.axon_site/_ro/trn_rl_repo"):
    if os.path.isdir(_p) and _p not in sys.path:
        sys.path.insert(0, _p)

from dataclasses import dataclass

import ml_dtypes
import numpy as np

import concourse.bacc as bacc
import concourse.bass as bass
import concourse.tile as tile
from concourse import mybir
from concourse.bass_utils import run_bass_kernel_spmd

P = 128
BF16 = mybir.dt.bfloat16
F32 = mybir.dt.float32
I16 = mybir.dt.int16
AL = mybir.AluOpType
AF = mybir.ActivationFunctionType

NEG_SLOPE_ATT = 0.2
NEG_SLOPE_ACT = 0.01
LN_EPS = 1e-5


def _ceil(a, b):
    return -(-a // b)


def _pad_elem(n_f32_elems):
    """bf16 row length (elements) padded so row bytes are a multiple of 256."""
    return _ceil(n_f32_elems * 2, 256) * 128


@dataclass
class Cfg:
    N: int = 50000
    E: int = 400000
    F_IN: int = 256
    HEADS: int = 4
    C1: int = 256
    C2: int = 128
    NCLS: int = 32
    NCORES: int = 8

    def __post_init__(self):
        assert self.N % self.NCORES == 0
        self.NL = self.N // self.NCORES
        self.T = _ceil(self.NL, P)
        self.NLP = self.T * P
        self.NPTOT = self.NLP * self.NCORES
        # the full hs table is split into two HALF tensors (half-major
        # layout), each written by a single AllGather so the collectives
        # pipeline with the producing/consuming phases; int16 gather
        # indices are relative to the owning half.
        q = self.T // 2
        self.CHT = [self.T - q, q]
        self.CH_T0 = np.cumsum([0] + self.CHT).tolist()  # half start tile
        self.CH_ROWS = [c * P for c in self.CHT]
        self.CH_BASE = np.cumsum(
            [0] + [r * self.NCORES for r in self.CH_ROWS]).tolist()
        self.B = self.CH_BASE[1]
        assert self.B <= 32767 and self.NPTOT - self.B <= 32767
        H = self.HEADS
        self.CO1 = H * self.C1
        self.CO2 = H * self.C2
        assert self.F_IN % P == 0 and self.CO1 % P == 0 and self.CO2 % P == 0
        self.ELEM1 = _pad_elem(self.CO1 + H)
        self.ELEM2 = _pad_elem(self.CO2 + H)
        self.ELEM3 = _pad_elem(self.NCLS + 1)


@dataclass
class Meta:
    nch: list  # [T][2] chunk counts (common across cores)
    si: list   # [T][2] idx16 column offsets
    sc: list   # [T][2] dstloc column offsets
    SI: int
    SC: int


def host_prep(cfg: Cfg, x, edge_src, edge_dst,
              W1, a_src1, a_dst1, b1, ln1_g, ln1_b,
              W2, a_src2, a_dst2, b2, ln2_g, ln2_b,
              W3, a_src3, a_dst3, b3, ln3_g, ln3_b):
    """Build per-core input maps + the (common) chunk structure."""
    c = cfg
    bf = ml_dtypes.bfloat16

    # ---- append self loops, shard edges by destination core
    loops = np.arange(c.N, dtype=np.int64)
    src = np.concatenate([edge_src.astype(np.int64), loops])
    dst = np.concatenate([edge_dst.astype(np.int64), loops])

    dst_core = dst // c.NL
    dstloc = dst - dst_core * c.NL
    tile_id = dstloc // P

    # padded-global source row in CHUNK-MAJOR layout:
    # row(cc,t,rr) = CH_BASE[k] + cc*CH_ROWS[k] + (t-CH_T0[k])*P + rr
    s_core = src // c.NL
    s_loc = src % c.NL
    s_t = s_loc // P
    s_rr = s_loc - s_t * P
    chunk_of_tile = np.zeros(c.T, np.int64)
    for k in range(2):
        chunk_of_tile[c.CH_T0[k]:c.CH_T0[k + 1]] = k
    half = chunk_of_tile[s_t]
    ch_rows = np.asarray(c.CH_ROWS, np.int64)
    ch_t0 = np.asarray(c.CH_T0[:2], np.int64)
    idx16 = (s_core * ch_rows[half] + (s_t - ch_t0[half]) * P + s_rr)

    # group edges per (core, tile, half)
    counts = np.zeros((c.NCORES, c.T, 2), np.int64)
    np.add.at(counts, (dst_core, tile_id, half), 1)
    nch = _ceil_arr(counts.max(axis=0), P)  # [T,2] chunks
    si = np.zeros((c.T, 2), np.int64)
    sc = np.zeros((c.T, 2), np.int64)
    acc_si = acc_sc = 0
    for t in range(c.T):
        for h in range(2):
            si[t, h] = acc_si
            sc[t, h] = acc_sc
            acc_si += int(nch[t, h]) * (P // 16)
            acc_sc += int(nch[t, h])
    SI, SC = int(acc_si), int(acc_sc)
    meta = Meta(nch=nch.tolist(), si=si.tolist(), sc=sc.tolist(), SI=SI, SC=SC)

    # ---- per-core index / dstloc tables
    order = np.lexsort((half, tile_id, dst_core))
    src_s, half_s, t_s, core_s = (idx16[order], half[order], tile_id[order],
                                  dst_core[order])
    dstrel_s = (dstloc - tile_id * P)[order]

    starts = np.zeros((c.NCORES, c.T, 2), np.int64)
    run = 0
    for cc in range(c.NCORES):
        for t in range(c.T):
            for h in range(2):
                starts[cc, t, h] = run
                run += int(counts[cc, t, h])

    idx_tabs, eq_tabs, eqt_tabs = [], [], []
    jj = np.arange(P, dtype=np.int64)
    for cc in range(c.NCORES):
        itab = np.zeros((16, SI), np.int16)
        eqtab = np.zeros((P, SC * P), bf)    # [e, (k, j)] one-hot by dstrel
        eqttab = np.zeros((P, SC * P), bf)   # [j, (k, e)] transposed one-hot
        for t in range(c.T):
            for h in range(2):
                m = int(counts[cc, t, h])
                n = int(nch[t, h])
                if n == 0:
                    continue
                s0 = int(starts[cc, t, h])
                iv = np.zeros(n * P, np.int16)
                iv[:m] = src_s[s0:s0 + m].astype(np.int16)
                cols = int(si[t, h])
                blk = iv.reshape(n * P // 16, 16).T  # idx k -> [k%16, k//16]
                itab[:, cols:cols + n * (P // 16)] = blk
                dv = np.full(n * P, -1, np.int64)
                dv[:m] = dstrel_s[s0:s0 + m]
                dvk = dv.reshape(n, P)                        # [k, e]
                # eq[e, (k, j)] = (dstrel(k, e) == j)
                eq_blk = (dvk[:, :, None] == jj[None, None, :])  # [k, e, j]
                c0 = int(sc[t, h])
                eqtab[:, c0 * P:(c0 + n) * P] = np.ascontiguousarray(
                    eq_blk.transpose(1, 0, 2).reshape(P, n * P)).astype(bf)
                # eqT[j, (k, e)] = (dstrel(k, e) == j)
                eqttab[:, c0 * P:(c0 + n) * P] = np.ascontiguousarray(
                    eq_blk.transpose(2, 0, 1).reshape(P, n * P)).astype(bf)
        idx_tabs.append(np.tile(itab, (8, 1)))
        eq_tabs.append(eqtab)
        eqt_tabs.append(eqttab)

    # ---- weights (augmented with U = W.T @ a columns), bf16
    def aug(W, a_s, a_d, H, C):
        WT = W.T.astype(np.float64)                      # [Fin, H*C]
        U_s = np.zeros((WT.shape[0], H))
        U_d = np.zeros((WT.shape[0], H))
        for h in range(H):
            U_s[:, h] = WT[:, h * C:(h + 1) * C] @ a_s[h].astype(np.float64)
            U_d[:, h] = WT[:, h * C:(h + 1) * C] @ a_d[h].astype(np.float64)
        return np.concatenate([WT, U_s, U_d], axis=1).astype(bf)

    W1a = aug(W1, a_src1, a_dst1, c.HEADS, c.C1)   # [F_IN, CO1+2H]
    W2a = aug(W2, a_src2, a_dst2, c.HEADS, c.C2)   # [CO1, CO2+2H]
    W3a = aug(W3, a_src3, a_dst3, 1, c.NCLS)       # [CO2, NCLS+2]

    def bln(b, g, be, D):
        row = np.concatenate([b, g, be]).astype(np.float32)[None, :]
        return np.repeat(row, P, axis=0)           # [128, 3D]

    bln1 = bln(b1, ln1_g, ln1_b, c.CO1)
    bln2 = bln(b2, ln2_g, ln2_b, c.CO2)
    bln3 = bln(b3, ln3_g, ln3_b, c.NCLS)

    ident = np.eye(P, dtype=bf)

    in_maps = []
    for cc in range(c.NCORES):
        xl = np.zeros((c.NLP, c.F_IN), np.float32)
        xl[:c.NL] = x[cc * c.NL:(cc + 1) * c.NL]
        in_maps.append({
            "xT": np.ascontiguousarray(xl.T).astype(bf),
            "W1a": W1a, "W2a": W2a, "W3a": W3a,
            "bln1": bln1, "bln2": bln2, "bln3": bln3,
            "idx16": idx_tabs[cc], "eqtab": eq_tabs[cc],
            "eqttab": eqt_tabs[cc], "ident": ident,
        })
    return in_maps, meta


def _ceil_arr(a, b):
    return -(-a // b)


# --------------------------------------------------------------------------
# device program
# --------------------------------------------------------------------------

def build_nc(cfg: Cfg, meta: Meta, max_phase: int = 6):
    c = cfg
    H = c.HEADS
    nc = bacc.Bacc("TRN2", target_bir_lowering=False, debug=False,
                   num_devices=c.NCORES, enable_partition_id=False)

    # ---- I/O
    xT = nc.dram_tensor("xT", [c.F_IN, c.NLP], BF16, kind="ExternalInput").ap()
    W1a = nc.dram_tensor("W1a", [c.F_IN, c.CO1 + 2 * H], BF16, kind="ExternalInput").ap()
    W2a = nc.dram_tensor("W2a", [c.CO1, c.CO2 + 2 * H], BF16, kind="ExternalInput").ap()
    W3a = nc.dram_tensor("W3a", [c.CO2, c.NCLS + 2], BF16, kind="ExternalInput").ap()
    bln1 = nc.dram_tensor("bln1", [P, 3 * c.CO1], F32, kind="ExternalInput").ap()
    bln2 = nc.dram_tensor("bln2", [P, 3 * c.CO2], F32, kind="ExternalInput").ap()
    bln3 = nc.dram_tensor("bln3", [P, 3 * c.NCLS], F32, kind="ExternalInput").ap()
    idx16 = nc.dram_tensor("idx16", [P, meta.SI], I16, kind="ExternalInput").ap()
    eqtab = nc.dram_tensor("eqtab", [P, meta.SC * P], BF16,
                           kind="ExternalInput").ap()
    eqttab = nc.dram_tensor("eqttab", [P, meta.SC * P], BF16,
                            kind="ExternalInput").ap()
    ident = nc.dram_tensor("ident", [P, P], BF16, kind="ExternalInput").ap()
    y = nc.dram_tensor("y", [c.NLP, c.NCLS], F32, kind="ExternalOutput").ap()

    groups = [list(range(c.NCORES))]

    def ag_half(nc, hs_loc, hs_halves, k):
        """AllGather one half table (single writer per Shared tensor)."""
        r0 = c.CH_T0[k] * P
        rows = c.CH_ROWS[k]
        nc.gpsimd.collective_compute(
            "AllGather", AL.bypass, replica_groups=groups,
            ins=[hs_loc[r0:r0 + rows, :].opt()],
            outs=[hs_halves[k][:, :].opt()])

    with tile.TileContext(nc) as tc:
        # ---- persistent DRAM intermediates
        dram_cm = tc.tile_pool(name="dram", bufs=1, space="DRAM")
        dram = dram_cm.__enter__()
        aspace = "Shared" if c.NCORES > 4 else "Local"
        R0, R1 = c.B, c.NPTOT - c.B
        hs1_loc = dram.tile([c.NLP, c.ELEM1], BF16)
        hs1_h0 = dram.tile([R0, c.ELEM1], BF16, addr_space=aspace)
        hs1_h1 = dram.tile([R1, c.ELEM1], BF16, addr_space=aspace)
        hs1_h = [hs1_h0, hs1_h1]
        hs2_loc = dram.tile([c.NLP, c.ELEM2], BF16)
        hs2_h0 = dram.tile([R0, c.ELEM2], BF16, addr_space=aspace)
        hs2_h1 = dram.tile([R1, c.ELEM2], BF16, addr_space=aspace)
        hs2_h = [hs2_h0, hs2_h1]
        hs3_loc = dram.tile([c.NLP, c.ELEM3], BF16)
        hs3_h0 = dram.tile([R0, c.ELEM3], BF16, addr_space=aspace)
        hs3_h1 = dram.tile([R1, c.ELEM3], BF16, addr_space=aspace)
        hs3_h = [hs3_h0, hs3_h1]

        # ---- persistent SBUF constants
        cpool_cm = tc.tile_pool(name="const", bufs=1)
        cpool = cpool_cm.__enter__()
        KC1 = c.F_IN // P
        W1w = c.CO1 + 2 * H
        W1a_sb = cpool.tile([P, KC1 * W1w], BF16)
        for k in range(KC1):
            nc.sync.dma_start(W1a_sb[:, k * W1w:(k + 1) * W1w],
                              W1a[k * P:(k + 1) * P, :])
        KC2 = c.CO1 // P
        W2w = c.CO2 + 2 * H
        W2a_sb = cpool.tile([P, KC2 * W2w], BF16)
        for k in range(KC2):
            nc.sync.dma_start(W2a_sb[:, k * W2w:(k + 1) * W2w],
                              W2a[k * P:(k + 1) * P, :])
        KC3 = c.CO2 // P
        W3w = c.NCLS + 2
        W3a_sb = cpool.tile([P, KC3 * W3w], BF16)
        for k in range(KC3):
            nc.sync.dma_start(W3a_sb[:, k * W3w:(k + 1) * W3w],
                              W3a[k * P:(k + 1) * P, :])
        bln1_sb = cpool.tile([P, 3 * c.CO1], F32)
        nc.sync.dma_start(bln1_sb[:], bln1[:])
        bln2_sb = cpool.tile([P, 3 * c.CO2], F32)
        nc.sync.dma_start(bln2_sb[:], bln2[:])
        bln3_sb = cpool.tile([P, 3 * c.NCLS], F32)
        nc.sync.dma_start(bln3_sb[:], bln3[:])
        idx_sb = cpool.tile([P, meta.SI], I16)
        nc.sync.dma_start(idx_sb[:], idx16[:])
        id_sb = cpool.tile([P, P], BF16)
        nc.sync.dma_start(id_sb[:], ident[:])
        # per-tile destination attention logits (bf16), layer 1..3
        d1_sb = cpool.tile([P, c.T * H], BF16)
        d2_sb = cpool.tile([P, c.T * H], BF16)
        d3_sb = cpool.tile([P, c.T * 1], BF16)

        # ================= phase A: h1 = x @ W1a (per local node tile)
        with (
            tc.tile_pool(name="pA", bufs=3) as pA,
            tc.tile_pool(name="pAp", bufs=2, space="PSUM") as pAp,
        ):
            for t in range(c.T):
                xt = pA.tile([P, KC1 * P], BF16, tag="xt")
                for k in range(KC1):
                    nc.sync.dma_start(xt[:, k * P:(k + 1) * P],
                                      xT[k * P:(k + 1) * P, t * P:(t + 1) * P])
                hp = pAp.tile([P, W1w], F32, tag="hp")
                _mm_splits(nc, hp, xt, W1a_sb, KC1, W1w, P)
                _store_hs(nc, pA, hp, c.CO1, H, c.ELEM1, hs1_loc, d1_sb, t)

        if max_phase >= 1:
            ag_half(nc, hs1_loc, hs1_h, 0)
            ag_half(nc, hs1_loc, hs1_h, 1)

        # ================= layer-1 aggregation + LN + fused L2 matmul
        if max_phase >= 2:
            _edge_phase(
                nc, tc, c, meta, lay=1, Hn=H, Ch=c.C1, ELEM=c.ELEM1,
                hs_h=hs1_h, d_sb=d1_sb, bln_sb=bln1_sb,
                id_sb=id_sb, idx_sb=idx_sb, eqtab=eqtab, eqttab=eqttab,
                fuse=dict(W_sb=W2a_sb, KC=KC2, Ww=W2w, CO=c.CO2, Hn2=H,
                          ELEMn=c.ELEM2, hs_loc=hs2_loc, d_next=d2_sb,
                          hs_h_n=hs2_h, ag=ag_half if max_phase >= 3 else None),
                final=None, y=None)

        # ================= layer-2 aggregation + LN + fused L3 matmul
        if max_phase >= 4:
            _edge_phase(
                nc, tc, c, meta, lay=2, Hn=H, Ch=c.C2, ELEM=c.ELEM2,
                hs_h=hs2_h, d_sb=d2_sb, bln_sb=bln2_sb,
                id_sb=id_sb, idx_sb=idx_sb, eqtab=eqtab, eqttab=eqttab,
                fuse=dict(W_sb=W3a_sb, KC=KC3, Ww=W3w, CO=c.NCLS, Hn2=1,
                          ELEMn=c.ELEM3, hs_loc=hs3_loc, d_next=d3_sb,
                          hs_h_n=hs3_h, ag=ag_half if max_phase >= 5 else None),
                final=None, y=None)

        # ================= layer-3 aggregation + LN + log_softmax
        if max_phase >= 6:
            _edge_phase(
                nc, tc, c, meta, lay=3, Hn=1, Ch=c.NCLS, ELEM=c.ELEM3,
                hs_h=hs3_h, d_sb=d3_sb, bln_sb=bln3_sb,
                id_sb=id_sb, idx_sb=idx_sb, eqtab=eqtab, eqttab=eqttab,
                fuse=None, final=True, y=y)

        cpool_cm.__exit__(None, None, None)
        dram_cm.__exit__(None, None, None)

    nc.compile()
    return nc


def _mm_splits(nc, out_ps, lhs_sb, w_sb, KC, Ww, Plhs):
    """out_ps[:, :Ww] = sum_k lhs_k.T @ W_k, with N split at 512."""
    splits = []
    n0 = 0
    while n0 < Ww:
        nsz = min(512, Ww - n0)
        splits.append((n0, nsz))
        n0 += nsz
    for k in range(KC):
        for (n0, nsz) in splits:
            nc.tensor.matmul(
                out=out_ps[:, n0:n0 + nsz],
                lhsT=lhs_sb[:, k * Plhs:(k + 1) * Plhs],
                rhs=w_sb[:, k * Ww + n0:k * Ww + n0 + nsz],
                start=(k == 0), stop=(k == KC - 1))


def _store_hs(nc, pool, hp, CO, Hn, ELEM, hs_loc, d_sb, t):
    """PSUM [128, CO+2H] -> bf16 hs row tile; d column -> persistent SBUF."""
    hst = pool.tile([P, ELEM], BF16, tag="hst")
    nc.scalar.copy(hst[:, 0:CO + Hn], hp[:, 0:CO + Hn])
    nc.vector.tensor_copy(d_sb[:, t * Hn:(t + 1) * Hn],
                          hp[:, CO + Hn:CO + 2 * Hn])
    nc.sync.dma_start(hs_loc[t * P:(t + 1) * P, :], hst[:])


def _bcast3(ap, n_mid):
    """[P, X] AP -> [P, n_mid, X] with stride-0 middle dim."""
    return bass.AP(ap.tensor, ap.offset,
                   [list(ap.ap[0]), [0, n_mid], list(ap.ap[1])])


def _edge_phase(nc, tc, c: Cfg, meta: Meta, lay, Hn, Ch, ELEM, hs_h, d_sb,
                bln_sb, id_sb, idx_sb, eqtab, eqttab,
                fuse, final, y):
    CO = Hn * Ch
    max_nch = max(max(r) for r in meta.nch)
    max_ntot = max(r[0] + r[1] for r in meta.nch)
    merge_den = (Hn == 1)
    # next-layer half boundaries: tile index -> half id to all-gather
    ag_after = {c.CH_T0[k + 1] - 1: k for k in range(2)} if (
        fuse and fuse.get("ag")) else {}

    with (
        tc.tile_pool(name=f"sb{lay}", bufs=2) as sb,
        tc.tile_pool(name=f"sc{lay}", bufs=4) as sbc,
        tc.tile_pool(name=f"sq{lay}", bufs=2) as sbq,
        tc.tile_pool(name=f"g{lay}", bufs=4) as gp,
        tc.tile_pool(name=f"ps{lay}", bufs=1, space="PSUM") as ps1,
        tc.tile_pool(name=f"psagg{lay}", bufs=2, space="PSUM") as psA,
    ):
        for t in range(c.T):
            nch0, nch1 = meta.nch[t]
            ntot = nch0 + nch1
            agg = psA.tile([P, CO + (1 if merge_den else 0)], F32, tag="agg")
            # dd: [dvals(ntot*Hn) | den(Hn)] in one PSUM bank
            dd = ps1.tile([P, (max_ntot + 1) * Hn], F32, tag="dd")
            den_ap = agg[:, CO:CO + 1] if merge_den else dd[:, ntot * Hn:
                                                            (ntot + 1) * Hn]

            c0 = meta.sc[t][0]
            eqa = sbq.tile([P, max_ntot * P], BF16, tag="eqa")
            nc.sync.dma_start(eqa[:, 0:ntot * P],
                              eqtab[:, c0 * P:(c0 + ntot) * P])
            eqT = sbq.tile([P, max_ntot * P], BF16, tag="eqT")
            nc.sync.dma_start(eqT[:, 0:ntot * P],
                              eqttab[:, c0 * P:(c0 + ntot) * P])

            Gs = []
            for hf, nch in ((0, nch0), (1, nch1)):
                if nch == 0:
                    Gs.append(None)
                    continue
                G = gp.tile([P, max_nch * ELEM], BF16, tag="G")
                si = meta.si[t][hf]
                nidx = nch * P
                nc.gpsimd.dma_gather(
                    out_ap=G[:, 0:nch * ELEM].rearrange(
                        "p (k d) -> p k d", d=ELEM),
                    in_ap=hs_h[hf][:, :],
                    idxs_ap=idx_sb[:, si:si + nch * (P // 16)],
                    num_idxs=nidx, num_idxs_reg=nidx, elem_size=ELEM)
                Gs.append(G)

            # ---- per-edge dst logits via PE: dd[:, b*Hn:(b+1)*Hn] = eqT_b^T @ d
            # dvals form a closed accumulation group (PSUM is only readable
            # after the group's stop); den later opens a second group in the
            # same bank — safe because den's rhs (wfb) depends on tsda which
            # consumed the dvals first.
            dt_ap = d_sb[:, t * Hn:(t + 1) * Hn]
            for b in range(ntot):
                nc.tensor.matmul(
                    out=dd[:, b * Hn:(b + 1) * Hn],
                    lhsT=eqT[:, b * P:(b + 1) * P], rhs=dt_ap,
                    start=(b == 0),
                    stop=(b == ntot - 1))

            # ---- tsd = s (from G) + d; leaky; exp
            tsda = sbc.tile([P, max_ntot * Hn], F32, tag="tsda")
            for hf, nch in ((0, nch0), (1, nch1)):
                if nch == 0:
                    continue
                b0 = 0 if hf == 0 else nch0
                Gv = Gs[hf][:, 0:nch * ELEM].rearrange(
                    "p (k d) -> p k d", d=ELEM)[:, :, CO:CO + Hn]
                Dv = dd[:, b0 * Hn:(b0 + nch) * Hn].rearrange(
                    "p (k h) -> p k h", h=Hn)
                nc.vector.tensor_tensor(
                    out=tsda[:, b0 * Hn:(b0 + nch) * Hn].rearrange(
                        "p (k h) -> p k h", h=Hn),
                    in0=Gv, in1=Dv, op=AL.add)
            lra = sbc.tile([P, max_ntot * Hn], F32, tag="lra")
            nc.vector.scalar_tensor_tensor(
                out=lra[:, 0:ntot * Hn], in0=tsda[:, 0:ntot * Hn],
                scalar=NEG_SLOPE_ATT, in1=tsda[:, 0:ntot * Hn],
                op0=AL.mult, op1=AL.max)
            wfa = sbc.tile([P, max_ntot * Hn], F32, tag="wfa")
            nc.scalar.activation(wfa[:, 0:ntot * Hn], lra[:, 0:ntot * Hn],
                                 AF.Exp)
            wfb = sbc.tile([P, max_ntot * Hn], BF16, tag="wfb")
            nc.vector.tensor_copy(wfb[:, 0:ntot * Hn], wfa[:, 0:ntot * Hn])

            # ---- S_h = eq * wf_h  (one batched op per head)
            S = sbq.tile([P, Hn * max_ntot * P], BF16, tag="S")
            wfa_ap = wfa[:, 0:ntot * Hn]
            for h in range(Hn):
                wf_h = bass.AP(wfa_ap.tensor, wfa_ap.offset + h,
                               [list(wfa_ap.ap[0]), [Hn, ntot], [0, P]])
                nc.vector.tensor_tensor(
                    out=S[:, h * ntot * P:(h + 1) * ntot * P].rearrange(
                        "p (k d) -> p k d", d=P),
                    in0=eqa[:, 0:ntot * P].rearrange("p (k d) -> p k d", d=P),
                    in1=wf_h, op=AL.mult)

            # ---- aggregation matmuls
            first = True
            first_den = True
            gchunk = 0
            BK = 512  # f32 elems per psum bank
            for hf, nch in ((0, nch0), (1, nch1)):
                G = Gs[hf]
                for b in range(nch):
                    last = (gchunk == ntot - 1)
                    eq = eqa[:, gchunk * P:(gchunk + 1) * P]
                    for h in range(Hn):
                        h_first = (h * Ch) % BK == 0
                        h_last = ((h + 1) * Ch) % BK == 0 or (
                            h == Hn - 1 and not merge_den)
                        nc.tensor.matmul(
                            out=agg[:, h * Ch:(h + 1) * Ch],
                            lhsT=S[:, h * ntot * P + gchunk * P:
                                   h * ntot * P + (gchunk + 1) * P],
                            rhs=G[:, b * ELEM + h * Ch:b * ELEM + (h + 1) * Ch],
                            start=first and h_first, stop=last and h_last)
                    if merge_den:
                        nc.tensor.matmul(out=den_ap, lhsT=eq,
                                         rhs=wfb[:, gchunk:gchunk + 1],
                                         start=False, stop=last)
                    else:
                        nc.tensor.matmul(
                            out=den_ap, lhsT=eq,
                            rhs=wfb[:, gchunk * Hn:(gchunk + 1) * Hn],
                            start=first_den, stop=last)
                        first_den = False
                    first = False
                    gchunk += 1

            _epilogue(nc, sb, ps1, c, meta, lay, t, agg, den_ap, Hn, Ch, CO,
                      bln_sb, id_sb, fuse, final, y)
            if t in ag_after:
                fuse["ag"](nc, fuse["hs_loc"], fuse["hs_h_n"], ag_after[t])


def _epilogue(nc, sb, ps1, c, meta, lay, t, agg, den_ap, Hn, Ch, CO,
              bln_sb, id_sb, fuse, final, y):
    # out = agg / den per head; + bias; LN; (leaky + next matmul) | logsoftmax
    denr = sb.tile([P, Hn], F32, tag="denr")
    nc.vector.tensor_scalar(out=denr[:], in0=den_ap, scalar1=1e-16,
                            scalar2=None, op0=AL.add)
    rec = sb.tile([P, Hn], F32, tag="rec")
    nc.vector.reciprocal_approx_fast(rec[:], denr[:])
    ob = sb.tile([P, CO], F32, tag="ob")
    if Hn == 1:
        nc.vector.scalar_tensor_tensor(
            out=ob[:], in0=agg[:, 0:CO], scalar=rec[:, 0:1],
            in1=bln_sb[:, 0:CO], op0=AL.mult, op1=AL.add)
    else:
        o = sb.tile([P, CO], F32, tag="o")
        rap = rec[:]
        rec_b = bass.AP(rap.tensor, rap.offset,
                        [list(rap.ap[0]), [1, Hn], [0, Ch]])
        nc.vector.tensor_tensor(
            out=o[:].rearrange("p (h d) -> p h d", h=Hn),
            in0=agg[:, 0:CO].rearrange("p (h d) -> p h d", h=Hn),
            in1=rec_b, op=AL.mult)
        nc.vector.tensor_tensor(out=ob[:], in0=o[:], in1=bln_sb[:, 0:CO],
                                op=AL.add)
    # LayerNorm
    rs = sb.tile([P, 1], F32, tag="rs")
    nc.vector.tensor_reduce(out=rs[:], in_=ob[:], axis=mybir.AxisListType.X,
                            op=AL.add)
    nm = sb.tile([P, 1], F32, tag="nm")
    nc.vector.tensor_scalar(out=nm[:], in0=rs[:], scalar1=-1.0 / CO,
                            scalar2=None, op0=AL.mult)
    xc = sb.tile([P, CO], F32, tag="xc")
    nc.vector.tensor_scalar(out=xc[:], in0=ob[:], scalar1=nm[:, 0:1],
                            scalar2=None, op0=AL.add)
    sq = sb.tile([P, CO], F32, tag="sq")
    vs = sb.tile([P, 1], F32, tag="vs")
    nc.scalar.activation(sq[:], xc[:], AF.Square, accum_out=vs[:])
    vstd = sb.tile([P, 1], F32, tag="vstd")
    nc.vector.tensor_scalar(out=vstd[:], in0=vs[:], scalar1=1.0 / CO,
                            scalar2=LN_EPS, op0=AL.mult, op1=AL.add)
    rv = sb.tile([P, 1], F32, tag="rv")
    nc.vector.reciprocal_approx_fast(rv[:], vstd[:])
    rstd = sb.tile([P, 1], F32, tag="rstd")
    nc.scalar.activation(rstd[:], rv[:], AF.Sqrt)
    y1 = sb.tile([P, CO], F32, tag="y1")
    nc.vector.scalar_tensor_tensor(
        out=y1[:], in0=xc[:], scalar=rstd[:, 0:1],
        in1=bln_sb[:, CO:2 * CO], op0=AL.mult, op1=AL.mult)
    y2 = sb.tile([P, CO], F32, tag="y2")
    nc.vector.tensor_tensor(out=y2[:], in0=y1[:], in1=bln_sb[:, 2 * CO:3 * CO],
                            op=AL.add)

    if final:
        # log_softmax over CO, write y
        mx = sb.tile([P, 1], F32, tag="mx")
        nc.vector.tensor_reduce(out=mx[:], in_=y2[:],
                                axis=mybir.AxisListType.X, op=AL.max)
        nmx = sb.tile([P, 1], F32, tag="nmx")
        nc.vector.tensor_scalar(out=nmx[:], in0=mx[:], scalar1=-1.0,
                                scalar2=None, op0=AL.mult)
        xs = sb.tile([P, CO], F32, tag="xs")
        nc.vector.tensor_scalar(out=xs[:], in0=y2[:], scalar1=nmx[:, 0:1],
                                scalar2=None, op0=AL.add)
        ex = sb.tile([P, CO], F32, tag="ex")
        se = sb.tile([P, 1], F32, tag="se")
        nc.scalar.activation(ex[:], xs[:], AF.Exp, accum_out=se[:])
        lse = sb.tile([P, 1], F32, tag="lse")
        nc.scalar.activation(lse[:], se[:], AF.Ln)
        nlse = sb.tile([P, 1], F32, tag="nlse")
        nc.vector.tensor_scalar(out=nlse[:], in0=lse[:], scalar1=-1.0,
                                scalar2=None, op0=AL.mult)
        yo = sb.tile([P, CO], F32, tag="yo")
        nc.vector.tensor_scalar(out=yo[:], in0=xs[:], scalar1=nlse[:, 0:1],
                                scalar2=None, op0=AL.add)
        nc.sync.dma_start(y[t * P:(t + 1) * P, :], yo[:])
        return

    # leaky(0.01) -> bf16 x_next; fused next-layer matmul
    x2 = sb.tile([P, CO], BF16, tag="x2")
    nc.vector.scalar_tensor_tensor(
        out=x2[:], in0=y2[:], scalar=NEG_SLOPE_ACT, in1=y2[:],
        op0=AL.mult, op1=AL.max)
    W_sb, KC, Ww = fuse["W_sb"], fuse["KC"], fuse["Ww"]
    CO2, Hn2, ELEMn = fuse["CO"], fuse["Hn2"], fuse["ELEMn"]
    xt2 = sb.tile([P, KC * P], BF16, tag="xt2")
    for k in range(KC):
        scr = ps1.tile([P, P], BF16, tag="scr")
        nc.tensor.transpose(out=scr[:], in_=x2[:, k * P:(k + 1) * P],
                            identity=id_sb[:])
        nc.scalar.copy(xt2[:, k * P:(k + 1) * P], scr[:])
    hp = ps1.tile([P, Ww], F32, tag="hnext")
    _mm_splits(nc, hp, xt2, W_sb, KC, Ww, P)
    _store_hs(nc, sb, hp, CO2, Hn2, ELEMn, fuse["hs_loc"], fuse["d_next"], t)


# --------------------------------------------------------------------------
# entry point
# --------------------------------------------------------------------------

_CACHE = {}


def _get_nc(cfg, meta):
    key = (tuple(sorted((k, v) for k, v in cfg.__dict__.items()
                        if isinstance(v, (int, float, str)))),
           tuple(tuple(r) for r in meta.nch))
    if key not in _CACHE:
        _CACHE[key] = build_nc(cfg, meta)
    return _CACHE[key]


def kernel(**inputs):
    inputs = {k: np.asarray(v) for k, v in inputs.items()}
    x = inputs["x"]
    cfg = Cfg(N=x.shape[0], E=inputs["edge_src"].shape[0], F_IN=x.shape[1],
              HEADS=inputs["a_src1"].shape[0], C1=inputs["a_src1"].shape[1],
              C2=inputs["a_src2"].shape[1], NCLS=inputs["W3"].shape[0],
              NCORES=8)
    in_maps, meta = host_prep(cfg, **inputs)
    nc = _get_nc(cfg, meta)
    trace = bool(int(os.environ.get("GAT_TRACE", "0")))
    res = run_bass_kernel_spmd(nc, in_maps, core_ids=list(range(cfg.NCORES)),
                               trace=trace)
    global LAST_EXEC_NS
    LAST_EXEC_NS = res.exec_time_ns
    out = np.concatenate(
        [res.results[cc]["y"][:cfg.NL] for cc in range(cfg.NCORES)], axis=0)
    return out.astype(np.float32)


LAST_EXEC_NS = None


if __name__ == "__main__":
    pass
